# revision 102
# baseline (speedup 1.0000x reference)
"""AttnBlock (GroupNorm -> single-head attention over H*W -> proj -> residual)
for Trainium2, 8 NeuronCores via SPMD -- bf16-ingest fp8 DoubleRow edition.

Sharding: core = b*4 + qi (b = batch 0/1, qi = query-quarter 0..3). The host
rotates each core's x columns so its query quarter is always columns [0, NQ)
(softmax reduces over keys, so key order is irrelevant).

x and the three weight matrices stream in as bf16 (halves the HBM prologue
that gates GroupNorm stats and therefore every matmul). All large matmuls
(scores, O=V*P, V projection, softmax denominator z, output projection) run
as fp8e4m3 DoubleRow (0.5 cyc/row); the qk projection runs bf16 (1 cyc/row)
because fp8 there squares the score noise. Softmax uses a global shift folded
into the Exp activation bias with a x16 premultiplier keeping P in fp8 normal
range; the x64 scaling of the fp8 qk operand is divided back out by the Exp
scale. GroupNorm scale folds into the bf16 wqk scaling (input side) and the
qk cast scale/bias + wv8 cast (output/keys side), so the fp8 x-pack is
stats-free and overlaps the x DMA.

Schedule: x tiles land h-major; bn_stats chunks chase the DMA on DVE and the
fp8 x-pack chases it on Pool (3 tiles on ACT). The stats -> rstd -> scol
chain resolves ~1.5us after the last tile; wqk lands right behind x, and the
per-i-block qk projection + scores stream starts immediately. V projection,
z (DoubleRow ones-matmul), O accumulation and the output projection pipeline
through the scores stream exactly as in the f32 edition, with V-tile drains
on DVE and the trailing per-i-block work drained into PE idle slots.
"""
import sys

sys.path.insert(0, '/opt/trn_rl_repo')

import numpy as np

C = 512
NG = 32
EPS = 1e-6
B = 2
N = 4096          # H*W
NQ = 1024         # query quarter per core
NCT = 4           # C // 128
EXPC = 5.5        # global softmax shift
QKS = 64.0        # fp8 qk pre-scale
EBIAS = -EXPC + float(np.log(16.0))   # exp bias: e^(s - 5.5 + ln16)

_cache = {}


def _legalize_waits(nc, mybir):
    """Codegen allows exactly ONE sync wait per instruction. Hoist excess
    waits onto preceding same-engine NoOps (semantics preserving)."""
    gen = 0
    for f in nc.m.functions:
        for bb in f.blocks:
            insts = list(bb.instructions)
            out = []
            changed = False
            for inst in insts:
                si = inst.sync_info
                waits = list(si.on_wait) if si and si.on_wait else []
                if len(waits) > 1:
                    for w in waits[:-1]:
                        gen += 1
                        nop = mybir.InstNoOp(
                            name=f"waitnop_{gen}", ins=[], outs=[],
                            engine=inst.engine)
                        nop.sync_info = mybir.SyncInfo(on_wait=[w], on_update=[])
                        out.append(nop)
                    inst.sync_info = mybir.SyncInfo(
                        on_wait=[waits[-1]],
                        on_update=list(si.on_update) if si and si.on_update else [])
                    changed = True
                out.append(inst)
            if changed:
                bb.instructions = out


def _build():
    import concourse.bass as bass
    import concourse.tile as tile
    from concourse import mybir
    from contextlib import ExitStack

    f32r = mybir.dt.float32r
    f32 = mybir.dt.float32
    bf16 = mybir.dt.bfloat16
    f8 = mybir.dt.float8e4
    AF = mybir.ActivationFunctionType
    DR = mybir.MatmulPerfMode.DoubleRow
    MUL = mybir.AluOpType.mult
    ADD = mybir.AluOpType.add

    nc = bass.Bass(trn_type="TRN2", target_bir_lowering=False, debug=False)

    x = nc.dram_tensor("x", [C, N], bf16, kind="ExternalInput").ap()
    wqk = nc.dram_tensor("wqk", [C, C], bf16, kind="ExternalInput").ap()
    wvT = nc.dram_tensor("wvT", [C, C], bf16, kind="ExternalInput").ap()
    woT = nc.dram_tensor("woT", [C, C], bf16, kind="ExternalInput").ap()
    cpack = nc.dram_tensor("cpack", [128, 44], f32, kind="ExternalInput").ap()
    bmask = nc.dram_tensor("bmask", [8, 128], f32, kind="ExternalInput").ap()
    out = nc.dram_tensor("out", [C, NQ], f32, kind="ExternalOutput").ap()

    dma = nc.sync.dma_start
    dmap = nc.gpsimd.dma_start
    dmaa = nc.scalar.dma_start

    with tile.TileContext(nc) as tc, ExitStack() as top:
        xpool = top.enter_context(tc.tile_pool(name="xpool", bufs=1))
        consts = top.enter_context(tc.tile_pool(name="consts", bufs=1))
        wpool = top.enter_context(tc.tile_pool(name="wpool", bufs=1))
        xq8p = top.enter_context(tc.tile_pool(name="xq8p", bufs=1))
        qkp = top.enter_context(tc.tile_pool(name="qkp", bufs=1))
        vpool = top.enter_context(tc.tile_pool(name="vpool", bufs=1))
        ptp = top.enter_context(tc.tile_pool(name="ptp", bufs=48))
        spool = top.enter_context(tc.tile_pool(name="spool", bufs=1))
        osbp = top.enter_context(tc.tile_pool(name="osbp", bufs=8))
        rpool = top.enter_context(tc.tile_pool(name="rpool", bufs=4))
        outp = top.enter_context(tc.tile_pool(name="outp", bufs=16))
        ps_big = top.enter_context(tc.tile_pool(name="ps_big", bufs=4, space="PSUM"))
        ps_v = top.enter_context(tc.tile_pool(name="ps_v", bufs=2, space="PSUM"))
        ps_o = top.enter_context(tc.tile_pool(name="ps_o", bufs=1, space="PSUM"))
        ps_z = top.enter_context(tc.tile_pool(name="ps_z", bufs=1, space="PSUM"))

        # ---- consts: one packed DMA + bmask (Pool SWDGE queue: keeps the
        # HWDGE pipe clear for the x stream) ----
        cp = consts.tile([128, 44], f32r, tag="cp", name="cp")
        dmap(cp[:], cpack[:, :].bitcast(f32r))
        bm = consts.tile([8, 128], f32r, tag="bm", name="bm")
        dmap(bm[:], bmask.bitcast(f32r))
        gm = cp[:, 0:8]
        gm0 = cp[:, 28:36]
        gm75 = cp[:, 36:44]
        gam4 = cp[:, 8:12].bitcast(f32)
        bet4 = cp[:, 12:16].bitcast(f32)
        hqc = [cp[:, 16 + i:17 + i].bitcast(f32) for i in range(NCT)]
        bvc = [cp[:, 20 + i:21 + i].bitcast(f32) for i in range(NCT)]
        boc = [cp[:, 24 + i:25 + i].bitcast(f32) for i in range(NCT)]

        epst = consts.tile([128, 1], f32, tag="epst", name="epst")
        nc.vector.memset(epst[:], EPS)
        ebias = consts.tile([128, 1], f32, tag="ebias", name="ebias")
        nc.vector.memset(ebias[:], EBIAS)
        # z-ones are 16.0: they exactly cancel the x16 pre-scale on wv8
        # (kept out of e4m3 subnormal range), since o/z is scale-invariant
        ones8 = consts.tile([128, 2, 128], f8, tag="ones8", name="ones8")
        nc.vector.memset(ones8[:, :, :], 16.0)

        # prime the PE p-state clock: one tiny DR matmul right at t~0 so the
        # >3us ramp has elapsed by the time the real matmul stream starts
        prm = ps_z.tile([128, 2], f32, tag="zt", name="prm")
        nc.tensor.matmul(prm[:], ones8[:, :, 0:128], ones8[:, :, 0:2],
                         start=True, stop=True, perf_mode=DR)

        # ---- x resident first (h-major so fp8-pack column blocks complete
        # early). Stats chase the DMA: DVE runs bn_stats for ci 1-3 while
        # ACT covers ci0 with a fused fp8-cast+sum pass plus a Square+sum
        # pass (its xp tiles ride along for free); Pool casts the rest.
        # The group-sum matmul accumulates per ci as aggregates land. ----
        x_t = [[xpool.tile([128, 1024], bf16, tag=f"x{ci}_{h}",
                           name=f"x{ci}_{h}") for h in range(4)]
               for ci in range(NCT)]
        xp = [xq8p.tile([128, 2, N], f8, tag=f"xp{s}", name=f"xp{s}")
              for s in range(2)]
        statsAll = spool.tile([128, 8], f32r, tag="stA", name="statsAll")
        mvt = [spool.tile([128, 2], f32, tag=f"mv{i}", name=f"mv{i}")
               for i in range(NCT)]
        stats3 = [spool.tile([128, 8, 6], f32, tag=f"st3{i}", name=f"st3{i}")
                  for i in range(1, NCT)]
        aS = spool.tile([128, 4, 2], f32r, tag="aS", name="aS")
        aB = spool.tile([128, 2, 2], f32r, tag="aB", name="aB")
        sqscr = spool.tile([128, 1024], bf16, tag="sqscr", name="sqscr")
        # one PSUM accumulation region per ci: overlapping accumulation
        # groups in one bank are illegal, and the four group-sum streams
        # overlap in time. ps_o / ps_v are idle during the prologue.
        ssums = [ps_o.tile([8, 2], f32, tag="o", name="ss0"),
                 ps_v.tile([8, 2], f32, tag="v", name="ss1"),
                 ps_v.tile([8, 2], f32, tag="v", name="ss2"),
                 ps_z.tile([8, 2], f32, tag="zt", name="ss3")]
        sg = spool.tile([8, 8], f32r, tag="sg", name="sg")
        vneg = spool.tile([8, 4], f32, tag="vneg", name="vneg")
        for h in range(4):
            for ci in (0, 1, 2, 3):
                dma(x_t[ci][h][:],
                    x[ci * 128:(ci + 1) * 128, h * 1024:(h + 1) * 1024])
                dst = xp[ci // 2][:, ci % 2, h * 1024:(h + 1) * 1024]
                if ci == 0:
                    with nc.allow_low_precision(reason="f32r group sums"):
                        nc.scalar.activation(dst, x_t[ci][h][:], AF.Copy,
                                             accum_out=aS[:, h, 0:1])
                        nc.scalar.activation(sqscr[:], x_t[ci][h][:],
                                             AF.Square,
                                             accum_out=aS[:, h, 1:2])
                    # raw-sum group aggregation rides the PE with a mask
                    # pre-scaled by 1/(16*4096); no DVE combine ops at all
                    nc.tensor.matmul(ssums[0][:, :], gm0, aS[:, h, :],
                                     start=(h == 0), stop=(h == 3))
                    if h == 3:
                        nc.vector.tensor_copy(sg[:, 0:2], ssums[0][:, :])
                        nc.vector.scalar_tensor_tensor(
                            out=vneg[:, 0:1], in0=sg[:, 0:1],
                            scalar=sg[:, 0:1], in1=sg[:, 1:2],
                            op0=MUL, op1=mybir.AluOpType.subtract)
                    continue
                if ci in (1, 2) and h == 0:
                    # the h0 tiles of ci1/ci2 ride ACT too, easing the DVE
                    # bn_stats backlog; their raw sums fold into ssum via gm0
                    # while the bn path for h1-3 gets a 3/4-scaled mask
                    with nc.allow_low_precision(reason="f32r group sums"):
                        nc.scalar.activation(dst, x_t[ci][h][:], AF.Copy,
                                             accum_out=aB[:, ci - 1, 0:1])
                        nc.scalar.activation(sqscr[:], x_t[ci][h][:],
                                             AF.Square,
                                             accum_out=aB[:, ci - 1, 1:2])
                    nc.tensor.matmul(ssums[ci][:, :], gm0,
                                     aB[:, ci - 1, :],
                                     start=True, stop=False)
                    continue
                for k in range(2):
                    nc.vector.bn_stats(
                        out=stats3[ci - 1][:, h * 2 + k, :],
                        in_=x_t[ci][h][:, k * 512:(k + 1) * 512])
                nc.gpsimd.tensor_copy(dst, x_t[ci][h][:])
                if h == 3:
                    mv = mvt[ci]
                    in3 = (stats3[ci - 1][:, 2:8, :] if ci in (1, 2)
                           else stats3[ci - 1][:, :, :])
                    nc.vector.bn_aggr(out=mv[:], in_=in3)
                    nc.vector.tensor_copy(statsAll[:, 2 * ci:2 * ci + 1],
                                          mv[:, 0:1])
                    # E[x^2] = mean^2 + var in one fused op
                    nc.vector.scalar_tensor_tensor(
                        out=statsAll[:, 2 * ci + 1:2 * ci + 2], in0=mv[:, 0:1],
                        scalar=mv[:, 0:1], in1=mv[:, 1:2], op0=MUL, op1=ADD)
                    nc.tensor.matmul(ssums[ci][:, :],
                                     gm75 if ci in (1, 2) else gm,
                                     statsAll[:, 2 * ci:2 * ci + 2],
                                     start=(ci not in (1, 2)), stop=True)
                    nc.vector.tensor_copy(sg[:, 2 * ci:2 * ci + 2],
                                          ssums[ci][:, :])
                    # -var = mean^2 - E[x^2], fused per ci as its slice lands
                    nc.vector.scalar_tensor_tensor(
                        out=vneg[:, ci:ci + 1], in0=sg[:, 2 * ci:2 * ci + 1],
                        scalar=sg[:, 2 * ci:2 * ci + 1],
                        in1=sg[:, 2 * ci + 1:2 * ci + 2],
                        op0=MUL, op1=mybir.AluOpType.subtract)

        # ---- weights (land right after x on the HBM queue; single DMA
        # each via a strided dram view) ----
        wqt = wpool.tile([128, 4, C], bf16, tag="wqt", name="wqt")
        dma(wqt[:, :, :], wqk.rearrange('(c p) m -> p c m', p=128))
        wvt = wpool.tile([128, 4, C], bf16, tag="wvt", name="wvt")
        dma(wvt[:, :, :], wvT.rearrange('(c p) m -> p c m', p=128))
        wot = wpool.tile([128, 4, C], bf16, tag="wot", name="wot")
        dma(wot[:, :, :], woT.rearrange('(c p) m -> p c m', p=128))
        wqk_t = [wqt[:, i, :] for i in range(NCT)]
        wv_t = [wvt[:, i, :] for i in range(NCT)]
        wo_t = [wot[:, i, :] for i in range(NCT)]

        # ---- P1: group stats -> per-channel scale s_col / shift t4 ----
        sqv = spool.tile([8, 4], f32, tag="sqv", name="sqv")
        nc.scalar.activation(sqv[:], vneg[:], AF.Sqrt, bias=epst[0:8, :],
                             scale=-1.0)
        with nc.allow_low_precision(reason="exact DVE divide"):
            nc.vector.reciprocal(sg[:, 1:8:2], sqv[:])
        # (gmean, rstd) broadcast to channel rows in one matmul; gamma is
        # host-folded into the wqk/wv rows and beta rides the host-folded
        # bias vectors. f32r moving must stay contiguous, hence interleaved.
        pc = ps_z.tile([128, 8], f32, tag="zt", name="pc")
        nc.tensor.matmul(pc[:, :], bm[:], sg[:, :], start=True, stop=True)
        # wqk scaled, bf16 (fp8 here would square the score noise); the
        # scale comes straight from the PSUM broadcast
        wqk_s = [wpool.tile([128, C], bf16, tag=f"wqs{i}", name=f"wqs{i}")
                 for i in range(NCT)]
        for ci in range(NCT):
            nc.vector.tensor_scalar_mul(wqk_s[ci][:], wqk_t[ci],
                                        pc[:, 2 * ci + 1:2 * ci + 2])
        rsb4 = consts.tile([128, 4], f32, tag="rsb4", name="rsb4")
        nc.vector.tensor_copy(rsb4[:], pc[:, 1:8:2])
        # t4 = -gmean*rstd (the y-form shift; beta terms are host-folded)
        t4 = consts.tile([128, 4], bf16, tag="t4", name="t4")
        nc.vector.scalar_tensor_tensor(out=t4[:], in0=pc[:, 0:8:2],
                                       scalar=-1.0,
                                       in1=rsb4[:], op0=MUL, op1=MUL)
        s64_4 = consts.tile([128, 4], f32, tag="s64_4", name="s64_4")
        nc.vector.scalar_tensor_tensor(out=s64_4[:], in0=rsb4[:], scalar=QKS,
                                       in1=gam4, op0=MUL, op1=MUL)
        s16_4 = consts.tile([128, 4], f32, tag="s16_4", name="s16_4")
        nc.vector.tensor_scalar_mul(s16_4[:], rsb4[:], 16.0)

        # ---- qk projection per i-block (bf16) + fp8 cast; the mt-sliced
        # PSUM lets casts chase the accumulation ----
        qk8 = [[qkp.tile([128, 2, 512], f8, tag=f"qk8_{s}_{ih}",
                         name=f"qk8_{s}_{ih}") for ih in range(2)]
               for s in range(2)]
        us4 = consts.tile([128, 4], f32, tag="us4", name="us4")
        su64_4 = consts.tile([128, 4], f32, tag="su64_4", name="su64_4")
        bvt4 = consts.tile([128, 4], bf16, tag="bvt4", name="bvt4")
        bos4 = consts.tile([128, 4], f32, tag="bos4", name="bos4")

        def gen_qk(ic, folds=False):
            qk_ps = [ps_big.tile([128, 2, 256], f32, tag="big",
                                 name=f"qkps{ic}_{half}") for half in range(2)]
            def cast_mt(mt):
                # ic0 casts on ACT (idle during the prologue; DVE still owns
                # the stats chain); later ics on DVE (truncating, but the
                # error headroom covers it) keeping ACT clear for exp
                dst = qk8[mt // 2][ic // 2][:, mt % 2,
                                            (ic % 2) * 256:(ic % 2) * 256 + 256]
                if ic == 0 and mt < 2:
                    nc.scalar.activation(
                        dst, qk_ps[mt // 2][:, mt % 2, :], AF.Identity,
                        scale=s64_4[:, mt:mt + 1], bias=su64_4[:, mt:mt + 1])
                else:
                    nc.vector.tensor_scalar(
                        dst, qk_ps[mt // 2][:, mt % 2, :],
                        s64_4[:, mt:mt + 1], su64_4[:, mt:mt + 1],
                        op0=MUL, op1=ADD)

            for mt in range(NCT):
                qkps = qk_ps[mt // 2][:, mt % 2, :]
                m_sl = slice(mt * 128, (mt + 1) * 128)
                for ct in range(NCT):
                    nc.tensor.matmul(qkps, wqk_s[ct][:, m_sl],
                                     x_t[ct][0][:, ic * 256:ic * 256 + 256],
                                     start=(ct == 0), stop=(ct == NCT - 1))
                if folds and mt == 0:
                    # P2 bias folds ride the PE stream right after the first
                    # qk tile: u = wqk^T t + hq (per out-channel), then
                    # bvt = wv^T t + bv. su64 = (u) * 64*scol gates the casts.
                    pq = ps_z.tile([128, 4], f32, tag="zt", name="pq")
                    for ot in range(NCT):
                        o_sl = slice(ot * 128, (ot + 1) * 128)
                        for ci in range(NCT):
                            nc.tensor.matmul(pq[:, ot:ot + 1],
                                             wqk_t[ci][:, o_sl],
                                             t4[:, ci:ci + 1],
                                             start=(ci == 0),
                                             stop=(ci == NCT - 1))
                    nc.vector.tensor_add(us4[:], pq[:, :],
                                         cp[:, 16:20].bitcast(f32))
                    nc.vector.tensor_mul(su64_4[:], us4[:], s64_4[:])
                    pv = ps_z.tile([128, 4], f32, tag="zt", name="pv")
                    for ot in range(NCT):
                        o_sl = slice(ot * 128, (ot + 1) * 128)
                        for ci in range(NCT):
                            nc.tensor.matmul(pv[:, ot:ot + 1],
                                             wv_t[ci][:, o_sl],
                                             t4[:, ci:ci + 1],
                                             start=(ci == 0),
                                             stop=(ci == NCT - 1))
                    bvf = spool.tile([128, 4], f32, tag="bvf", name="bvf")
                    nc.vector.tensor_add(bvf[:], pv[:, :],
                                         cp[:, 20:24].bitcast(f32))
                    nc.vector.tensor_copy(bvt4[:], bvf[:])
                yield
            for mt in range(NCT):
                cast_mt(mt)
                yield

        # wv -> fp8 packed with GN scale (x16 against subnormals) folded;
        # DVE so neither Pool (x-pack) nor ACT (casts+exp) stalls V
        wv8 = [wpool.tile([128, 2, C], f8, tag=f"wv8_{s}", name=f"wv8_{s}")
               for s in range(2)]

        def emit_wv8():
            for ci in range(NCT):
                if ci < 2:
                    nc.scalar.activation(wv8[ci // 2][:, ci % 2, :],
                                         wv_t[ci], AF.Identity,
                                         scale=s16_4[:, ci:ci + 1])
                else:
                    nc.vector.tensor_scalar_mul(wv8[ci // 2][:, ci % 2, :],
                                                wv_t[ci], s16_4[:, ci:ci + 1])

        # wo -> fp8 packed (raw weights; output projection runs DR)
        wo8 = [wpool.tile([128, 2, C], f8, tag=f"wo8_{s}", name=f"wo8_{s}")
               for s in range(2)]

        def emit_wo8():
            for ci in range(NCT):
                nc.gpsimd.tensor_copy(wo8[ci // 2][:, ci % 2, :], wo_t[ci])

        def emit_bo_fold():
            pb = ps_z.tile([128, 4], f32, tag="zt", name="pb")
            for ot in range(NCT):
                o_sl = slice(ot * 128, (ot + 1) * 128)
                for ci in range(NCT):
                    nc.tensor.matmul(pb[:, ot:ot + 1], wo_t[ci][:, o_sl],
                                     bvt4[:, ci:ci + 1],
                                     start=(ci == 0), stop=(ci == NCT - 1))
            nc.vector.tensor_add(bos4[:], pb[:, :], cp[:, 24:28].bitcast(f32))

        # ---- V projection (fp8 DR) on the 2-bank ps_v ring, one jt per
        # tile, interleaved into the scores streams; drains on DVE ----
        v8 = [vpool.tile([128, 2, C], f8, tag=f"v8_{p}", name=f"v8_{p}")
              for p in range(16)]

        def emit_vjt(jt):
            vt = ps_v.tile([128, 512], f32, tag="v", name=f"vt{jt}")
            for ch2 in range(2):
                for s in range(2):
                    nc.tensor.matmul(
                        vt[:, ch2 * 256:ch2 * 256 + 256],
                        xp[s][:, :, jt * 128:jt * 128 + 128],
                        wv8[s][:, :, ch2 * 256:ch2 * 256 + 256],
                        start=(s == 0), stop=(s == 1), perf_mode=DR)
            # drains on DVE: ACT is the exp pacer and must stay clear. DVE
            # truncates fp8 casts but V noise is fp8-quantization-dominated.
            nc.vector.tensor_copy(v8[jt // 2][:, jt % 2, :], vt[:, :])

        # ---- main attention loop over i-blocks of 256 ----
        pt8 = {}
        pending = []

        def drain(n):
            done = 0
            while pending and done < n:
                try:
                    next(pending[0])
                    done += 1
                except StopIteration:
                    pending.pop(0)

        def emit_scores(ic, with_v=(), per_quad=0):
            pt8[ic] = []
            vq = {q: [] for q in range(16)}
            for i, jt in enumerate(with_v):
                vq[min(15, 2 + i * 16 // len(with_v))].append(jt)
            for q in range(16):
                sq = ps_big.tile([128, 2, 256], f32, tag="big",
                                 name=f"sq{ic}_{q}")
                for jq in range(2):
                    jt = q * 2 + jq
                    for s in range(2):
                        nc.tensor.matmul(
                            sq[:, jq, :],
                            xp[s][:, :, jt * 128:jt * 128 + 128],
                            qk8[s][ic // 2][:, :, (ic % 2) * 256:
                                            (ic % 2) * 256 + 256],
                            start=(s == 0), stop=(s == 1), perf_mode=DR)
                pt = ptp.tile([128, 2, 256], f8, tag="pt", name=f"pt{ic}_{q}")
                nc.scalar.activation(pt[:, :, :], sq[:, :, :], AF.Exp,
                                     scale=1.0 / QKS, bias=ebias[:])
                pt8[ic].append(pt)
                for jt in vq[q]:
                    emit_vjt(jt)
                drain(per_quad)

        Rs = {}

        def gen_z(ic):
            """z chain -> R, yielded per quad for PE interleaving."""
            zt = ps_z.tile([128, 512], f32, tag="zt", name=f"zt{ic}")
            for q in range(16):
                while len(pt8.get(ic, ())) <= q:
                    yield   # scores for this quad not emitted yet
                nc.tensor.matmul(zt[:, 0:256], ones8[:, :, :],
                                 pt8[ic][q][:, :, :],
                                 start=(q == 0), stop=(q == 15), perf_mode=DR)
                if q % 2 == 1:
                    yield
            R = rpool.tile([128, 256], f32, tag="R", name=f"R{ic}")
            with nc.allow_low_precision(reason="exact DVE divide"):
                nc.vector.reciprocal(R[:], zt[:, 0:256])
            Rs[ic] = R

        def gen_ofin(ic):
            """O accumulation -> normalize -> output proj -> residual+store.
            Tiles rotate over ps_o / ps_v (free after V-projection) for ring
            depth; z keeps ps_z."""
            while ic not in Rs:
                yield   # z chain for this i-block not finished emitting
            R = Rs[ic]
            osb8 = [osbp.tile([128, 2, 256], f8, tag="osb",
                              name=f"osb{ic}_{s}") for s in range(2)]
            for mt in range(NCT):
                pool = ps_o if mt % 2 == 0 else ps_v
                ot_ps = pool.tile([128, 512], f32,
                                  tag="o" if mt % 2 == 0 else "v",
                                  name=f"o{ic}_{mt}")
                for p in range(16):
                    while len(pt8.get(ic, ())) <= p:
                        yield   # this quad's scores not emitted yet
                    nc.tensor.matmul(ot_ps[:, 0:256],
                                     v8[p][:, :, mt * 128:mt * 128 + 128],
                                     pt8[ic][p][:, :, :],
                                     start=(p == 0), stop=(p == 15),
                                     perf_mode=DR)
                    if p % 4 == 3:
                        yield
                nc.vector.tensor_mul(osb8[mt // 2][:, mt % 2, :],
                                     ot_ps[:, 0:256], R[:])
            for ot in range(NCT):
                o_sl = slice(ot * 128, (ot + 1) * 128)
                pool = ps_o if ot % 2 == 0 else ps_v
                ft = pool.tile([128, 512], f32,
                               tag="o" if ot % 2 == 0 else "v",
                               name=f"f{ic}_{ot}")
                for s in range(2):
                    nc.tensor.matmul(ft[:, 0:256], wo8[s][:, :, o_sl],
                                     osb8[s][:, :, :], start=(s == 0),
                                     stop=(s == 1), perf_mode=DR)
                ot_sb = outp.tile([128, 256], f32, tag="outsb",
                                  name=f"ot{ic}_{ot}")
                n0 = ic * 256
                h, off = n0 // 1024, n0 % 1024
                nc.vector.scalar_tensor_tensor(
                    out=ot_sb[:], in0=ft[:, 0:256], scalar=bos4[:, ot:ot + 1],
                    in1=x_t[ot][h][:, off:off + 256],
                    op0=ADD, op1=ADD)
                # spread final stores across queues: the tail DMAs otherwise
                # serialize behind one engine's issue overhead
                eng = [dma, dmaa, dma, dmaa][ot] if ic == 3 else dma
                eng(out[o_sl, ic * 256:ic * 256 + 256], ot_sb[:])
                yield

        # --- software-pipelined emission; the O/fin part of stream ic drains
        # one stream later than its z part so every v8 write is emitted
        # before any consumer ---
        for _ in gen_qk(0, folds=True):
            pass
        emit_wv8()
        emit_wo8()
        for _ in gen_qk(1):
            pass
        emit_scores(0, with_v=range(0, 10), per_quad=2)
        emit_bo_fold()
        pending.append(gen_z(0))
        pending.append(gen_qk(2))
        emit_scores(1, with_v=range(10, 32), per_quad=4)
        pending.append(gen_ofin(0))
        pending.append(gen_qk(3))
        pending.append(gen_z(1))
        pending.append(gen_ofin(1))
        emit_scores(2, per_quad=7)
        pending.append(gen_z(2))
        pending.append(gen_z(3))
        pending.append(gen_ofin(2))
        pending.append(gen_ofin(3))
        emit_scores(3, per_quad=8)
        drain(10 ** 9)

    _legalize_waits(nc, mybir)
    return nc


def kernel(**inputs):
    import ml_dtypes
    import concourse.bass  # noqa: F401
    from concourse.bass_utils import run_bass_kernel_spmd

    bft = ml_dtypes.bfloat16
    x = np.asarray(inputs["x"], dtype=np.float32)
    gamma = np.asarray(inputs["gamma"], np.float32)
    beta = np.asarray(inputs["beta"], np.float32)
    wq = np.asarray(inputs["wq"], np.float32)
    bq = np.asarray(inputs["bq"], np.float32)
    wk = np.asarray(inputs["wk"], np.float32)
    wv = np.asarray(inputs["wv"], np.float32)
    wo = np.asarray(inputs["wo"], np.float32)
    bv = np.asarray(inputs["bv"], np.float32)
    bo = np.asarray(inputs["bo"], np.float32)

    Bb, Cc, H, W = x.shape
    scale = Cc ** (-0.5)
    xf = x.reshape(Bb, Cc, H * W)

    wqk_raw = scale * (wq.T @ wk)
    hq = scale * (wk.T @ bq) + wqk_raw.T @ beta   # [C] (+ beta fold)
    wqk_h = np.ascontiguousarray(wqk_raw * gamma[:, None]).astype(bft)
    bvh = bv + wv @ beta                          # beta fold for V
    wvT = np.ascontiguousarray((wv * gamma[None, :]).T).astype(bft)
    woT = np.ascontiguousarray(wo.T).astype(bft)

    cpack = np.zeros((128, 44), np.float32)
    for p in range(128):
        cpack[p, p // 16] = 1.0 / 16.0            # gmask (x 1/16)
        cpack[p, 28 + p // 16] = 1.0 / (16.0 * 4096.0)   # raw-sum gmask
        cpack[p, 36 + p // 16] = 0.75 / 16.0      # 3/4-weighted gmask (ci1)
    cpack[:, 8:12] = gamma.reshape(NCT, 128).T
    cpack[:, 12:16] = beta.reshape(NCT, 128).T
    cpack[:, 16:20] = hq.reshape(NCT, 128).T
    cpack[:, 20:24] = bvh.reshape(NCT, 128).T
    cpack[:, 24:28] = bo.reshape(NCT, 128).T
    bmask = np.zeros((8, 128), np.float32)
    for p in range(128):
        bmask[p // 16, p] = 1.0

    if "nc" not in _cache:
        _cache["nc"] = _build()
    nc = _cache["nc"]

    in_maps = []
    for core in range(8):
        b, qi = core // 4, core % 4
        xb = xf[b]
        # rotate columns so this core's query quarter sits at columns [0, NQ)
        xrot = np.ascontiguousarray(
            np.concatenate([xb[:, qi * NQ:], xb[:, :qi * NQ]],
                           axis=1)).astype(bft)
        in_maps.append({
            "x": xrot, "wqk": wqk_h, "wvT": wvT, "woT": woT,
            "cpack": cpack, "bmask": bmask,
        })

    res = run_bass_kernel_spmd(nc, in_maps, core_ids=list(range(8)))
    outf = np.empty((Bb, Cc, H * W), np.float32)
    for core in range(8):
        b, qi = core // 4, core % 4
        outf[b][:, qi * NQ:(qi + 1) * NQ] = res.results[core]["out"]
    return outf.reshape(Bb, Cc, H, W)


# revision 103
# speedup vs baseline: 1.0031x; 1.0031x over previous
"""AttnBlock (GroupNorm -> single-head attention over H*W -> proj -> residual)
for Trainium2, 8 NeuronCores via SPMD -- bf16-ingest fp8 DoubleRow edition.

Sharding: core = b*4 + qi (b = batch 0/1, qi = query-quarter 0..3). The host
rotates each core's x columns so its query quarter is always columns [0, NQ)
(softmax reduces over keys, so key order is irrelevant).

x and the three weight matrices stream in as bf16 (halves the HBM prologue
that gates GroupNorm stats and therefore every matmul). All large matmuls
(scores, O=V*P, V projection, softmax denominator z, output projection) run
as fp8e4m3 DoubleRow (0.5 cyc/row); the qk projection runs bf16 (1 cyc/row)
because fp8 there squares the score noise. Softmax uses a global shift folded
into the Exp activation bias with a x16 premultiplier keeping P in fp8 normal
range; the x64 scaling of the fp8 qk operand is divided back out by the Exp
scale. GroupNorm scale folds into the bf16 wqk scaling (input side) and the
qk cast scale/bias + wv8 cast (output/keys side), so the fp8 x-pack is
stats-free and overlaps the x DMA.

Schedule: x tiles land h-major; bn_stats chunks chase the DMA on DVE and the
fp8 x-pack chases it on Pool (3 tiles on ACT). The stats -> rstd -> scol
chain resolves ~1.5us after the last tile; wqk lands right behind x, and the
per-i-block qk projection + scores stream starts immediately. V projection,
z (DoubleRow ones-matmul), O accumulation and the output projection pipeline
through the scores stream exactly as in the f32 edition, with V-tile drains
on DVE and the trailing per-i-block work drained into PE idle slots.
"""
import sys

sys.path.insert(0, '/opt/trn_rl_repo')

import numpy as np

C = 512
NG = 32
EPS = 1e-6
B = 2
N = 4096          # H*W
NQ = 1024         # query quarter per core
NCT = 4           # C // 128
EXPC = 5.5        # global softmax shift
QKS = 64.0        # fp8 qk pre-scale
EBIAS = -EXPC + float(np.log(16.0))   # exp bias: e^(s - 5.5 + ln16)

_cache = {}


def _legalize_waits(nc, mybir):
    """Codegen allows exactly ONE sync wait per instruction. Hoist excess
    waits onto preceding same-engine NoOps (semantics preserving)."""
    gen = 0
    for f in nc.m.functions:
        for bb in f.blocks:
            insts = list(bb.instructions)
            out = []
            changed = False
            for inst in insts:
                si = inst.sync_info
                waits = list(si.on_wait) if si and si.on_wait else []
                if len(waits) > 1:
                    for w in waits[:-1]:
                        gen += 1
                        nop = mybir.InstNoOp(
                            name=f"waitnop_{gen}", ins=[], outs=[],
                            engine=inst.engine)
                        nop.sync_info = mybir.SyncInfo(on_wait=[w], on_update=[])
                        out.append(nop)
                    inst.sync_info = mybir.SyncInfo(
                        on_wait=[waits[-1]],
                        on_update=list(si.on_update) if si and si.on_update else [])
                    changed = True
                out.append(inst)
            if changed:
                bb.instructions = out


def _build():
    import concourse.bass as bass
    import concourse.tile as tile
    from concourse import mybir
    from contextlib import ExitStack

    f32r = mybir.dt.float32r
    f32 = mybir.dt.float32
    bf16 = mybir.dt.bfloat16
    f8 = mybir.dt.float8e4
    AF = mybir.ActivationFunctionType
    DR = mybir.MatmulPerfMode.DoubleRow
    MUL = mybir.AluOpType.mult
    ADD = mybir.AluOpType.add

    nc = bass.Bass(trn_type="TRN2", target_bir_lowering=False, debug=False)

    x = nc.dram_tensor("x", [C, N], bf16, kind="ExternalInput").ap()
    wqk = nc.dram_tensor("wqk", [C, C], bf16, kind="ExternalInput").ap()
    wvT = nc.dram_tensor("wvT", [C, C], bf16, kind="ExternalInput").ap()
    woT = nc.dram_tensor("woT", [C, C], bf16, kind="ExternalInput").ap()
    cpack = nc.dram_tensor("cpack", [128, 44], f32, kind="ExternalInput").ap()
    bmask = nc.dram_tensor("bmask", [8, 128], f32, kind="ExternalInput").ap()
    out = nc.dram_tensor("out", [C, NQ], f32, kind="ExternalOutput").ap()

    dma = nc.sync.dma_start
    dmap = nc.gpsimd.dma_start
    dmaa = nc.scalar.dma_start

    with tile.TileContext(nc) as tc, ExitStack() as top:
        xpool = top.enter_context(tc.tile_pool(name="xpool", bufs=1))
        consts = top.enter_context(tc.tile_pool(name="consts", bufs=1))
        wpool = top.enter_context(tc.tile_pool(name="wpool", bufs=1))
        xq8p = top.enter_context(tc.tile_pool(name="xq8p", bufs=1))
        qkp = top.enter_context(tc.tile_pool(name="qkp", bufs=1))
        vpool = top.enter_context(tc.tile_pool(name="vpool", bufs=1))
        ptp = top.enter_context(tc.tile_pool(name="ptp", bufs=48))
        spool = top.enter_context(tc.tile_pool(name="spool", bufs=1))
        osbp = top.enter_context(tc.tile_pool(name="osbp", bufs=8))
        rpool = top.enter_context(tc.tile_pool(name="rpool", bufs=4))
        outp = top.enter_context(tc.tile_pool(name="outp", bufs=16))
        ps_big = top.enter_context(tc.tile_pool(name="ps_big", bufs=4, space="PSUM"))
        ps_v = top.enter_context(tc.tile_pool(name="ps_v", bufs=2, space="PSUM"))
        ps_o = top.enter_context(tc.tile_pool(name="ps_o", bufs=1, space="PSUM"))
        ps_z = top.enter_context(tc.tile_pool(name="ps_z", bufs=1, space="PSUM"))

        # ---- consts: one packed DMA + bmask (Pool SWDGE queue: keeps the
        # HWDGE pipe clear for the x stream) ----
        cp = consts.tile([128, 44], f32r, tag="cp", name="cp")
        dmap(cp[:], cpack[:, :].bitcast(f32r))
        bm = consts.tile([8, 128], f32r, tag="bm", name="bm")
        dmap(bm[:], bmask.bitcast(f32r))
        gm = cp[:, 0:8]
        gm0 = cp[:, 28:36]
        gm75 = cp[:, 36:44]
        gam4 = cp[:, 8:12].bitcast(f32)
        bet4 = cp[:, 12:16].bitcast(f32)
        hqc = [cp[:, 16 + i:17 + i].bitcast(f32) for i in range(NCT)]
        bvc = [cp[:, 20 + i:21 + i].bitcast(f32) for i in range(NCT)]
        boc = [cp[:, 24 + i:25 + i].bitcast(f32) for i in range(NCT)]

        epst = consts.tile([128, 1], f32, tag="epst", name="epst")
        nc.vector.memset(epst[:], EPS)
        ebias = consts.tile([128, 1], f32, tag="ebias", name="ebias")
        nc.vector.memset(ebias[:], EBIAS)
        # z-ones are 16.0: they exactly cancel the x16 pre-scale on wv8
        # (kept out of e4m3 subnormal range), since o/z is scale-invariant
        ones8 = consts.tile([128, 2, 128], f8, tag="ones8", name="ones8")
        nc.vector.memset(ones8[:, :, :], 16.0)

        # prime the PE p-state clock: one tiny DR matmul right at t~0 so the
        # >3us ramp has elapsed by the time the real matmul stream starts
        prm = ps_z.tile([128, 2], f32, tag="zt", name="prm")
        nc.tensor.matmul(prm[:], ones8[:, :, 0:128], ones8[:, :, 0:2],
                         start=True, stop=True, perf_mode=DR)

        # ---- x resident first (h-major so fp8-pack column blocks complete
        # early). Stats chase the DMA: DVE runs bn_stats for ci 1-3 while
        # ACT covers ci0 with a fused fp8-cast+sum pass plus a Square+sum
        # pass (its xp tiles ride along for free); Pool casts the rest.
        # The group-sum matmul accumulates per ci as aggregates land. ----
        x_t = [[xpool.tile([128, 1024], bf16, tag=f"x{ci}_{h}",
                           name=f"x{ci}_{h}") for h in range(4)]
               for ci in range(NCT)]
        xp = [xq8p.tile([128, 2, N], f8, tag=f"xp{s}", name=f"xp{s}")
              for s in range(2)]
        statsAll = spool.tile([128, 8], f32r, tag="stA", name="statsAll")
        mvt = [spool.tile([128, 2], f32, tag=f"mv{i}", name=f"mv{i}")
               for i in range(NCT)]
        stats3 = [spool.tile([128, 8, 6], f32, tag=f"st3{i}", name=f"st3{i}")
                  for i in range(1, NCT)]
        aS = spool.tile([128, 4, 2], f32r, tag="aS", name="aS")
        aB = spool.tile([128, 2, 2], f32r, tag="aB", name="aB")
        sqscr = spool.tile([128, 1024], bf16, tag="sqscr", name="sqscr")
        # one PSUM accumulation region per ci: overlapping accumulation
        # groups in one bank are illegal, and the four group-sum streams
        # overlap in time. ps_o / ps_v are idle during the prologue.
        ssums = [ps_o.tile([8, 2], f32, tag="o", name="ss0"),
                 ps_v.tile([8, 2], f32, tag="v", name="ss1"),
                 ps_v.tile([8, 2], f32, tag="v", name="ss2"),
                 ps_z.tile([8, 2], f32, tag="zt", name="ss3")]
        sg = spool.tile([8, 8], f32r, tag="sg", name="sg")
        vneg = spool.tile([8, 4], f32, tag="vneg", name="vneg")
        for h in range(4):
            for ci in (0, 1, 2, 3):
                dma(x_t[ci][h][:],
                    x[ci * 128:(ci + 1) * 128, h * 1024:(h + 1) * 1024])
                dst = xp[ci // 2][:, ci % 2, h * 1024:(h + 1) * 1024]
                if ci == 0:
                    with nc.allow_low_precision(reason="f32r group sums"):
                        nc.scalar.activation(dst, x_t[ci][h][:], AF.Copy,
                                             accum_out=aS[:, h, 0:1])
                        nc.scalar.activation(sqscr[:], x_t[ci][h][:],
                                             AF.Square,
                                             accum_out=aS[:, h, 1:2])
                    # raw-sum group aggregation rides the PE with a mask
                    # pre-scaled by 1/(16*4096); no DVE combine ops at all
                    nc.tensor.matmul(ssums[0][:, :], gm0, aS[:, h, :],
                                     start=(h == 0), stop=(h == 3))
                    if h == 3:
                        nc.vector.tensor_copy(sg[:, 0:2], ssums[0][:, :])
                        nc.vector.scalar_tensor_tensor(
                            out=vneg[:, 0:1], in0=sg[:, 0:1],
                            scalar=sg[:, 0:1], in1=sg[:, 1:2],
                            op0=MUL, op1=mybir.AluOpType.subtract)
                    continue
                if ci in (1, 2) and h == 0:
                    # the h0 tiles of ci1/ci2 ride ACT too, easing the DVE
                    # bn_stats backlog; their raw sums fold into ssum via gm0
                    # while the bn path for h1-3 gets a 3/4-scaled mask
                    with nc.allow_low_precision(reason="f32r group sums"):
                        nc.scalar.activation(dst, x_t[ci][h][:], AF.Copy,
                                             accum_out=aB[:, ci - 1, 0:1])
                        nc.scalar.activation(sqscr[:], x_t[ci][h][:],
                                             AF.Square,
                                             accum_out=aB[:, ci - 1, 1:2])
                    nc.tensor.matmul(ssums[ci][:, :], gm0,
                                     aB[:, ci - 1, :],
                                     start=True, stop=False)
                    continue
                for k in range(2):
                    nc.vector.bn_stats(
                        out=stats3[ci - 1][:, h * 2 + k, :],
                        in_=x_t[ci][h][:, k * 512:(k + 1) * 512])
                nc.gpsimd.tensor_copy(dst, x_t[ci][h][:])
                if h == 3:
                    mv = mvt[ci]
                    in3 = (stats3[ci - 1][:, 2:8, :] if ci in (1, 2)
                           else stats3[ci - 1][:, :, :])
                    nc.vector.bn_aggr(out=mv[:], in_=in3)
                    nc.vector.tensor_copy(statsAll[:, 2 * ci:2 * ci + 1],
                                          mv[:, 0:1])
                    # E[x^2] = mean^2 + var in one fused op
                    nc.vector.scalar_tensor_tensor(
                        out=statsAll[:, 2 * ci + 1:2 * ci + 2], in0=mv[:, 0:1],
                        scalar=mv[:, 0:1], in1=mv[:, 1:2], op0=MUL, op1=ADD)
                    nc.tensor.matmul(ssums[ci][:, :],
                                     gm75 if ci in (1, 2) else gm,
                                     statsAll[:, 2 * ci:2 * ci + 2],
                                     start=(ci not in (1, 2)), stop=True)
                    nc.vector.tensor_copy(sg[:, 2 * ci:2 * ci + 2],
                                          ssums[ci][:, :])
                    # -var = mean^2 - E[x^2], fused per ci as its slice lands
                    nc.vector.scalar_tensor_tensor(
                        out=vneg[:, ci:ci + 1], in0=sg[:, 2 * ci:2 * ci + 1],
                        scalar=sg[:, 2 * ci:2 * ci + 1],
                        in1=sg[:, 2 * ci + 1:2 * ci + 2],
                        op0=MUL, op1=mybir.AluOpType.subtract)

        # ---- weights (land right after x on the HBM queue; single DMA
        # each via a strided dram view) ----
        wqt = wpool.tile([128, 4, C], bf16, tag="wqt", name="wqt")
        dma(wqt[:, :, :], wqk.rearrange('(c p) m -> p c m', p=128))
        wvt = wpool.tile([128, 4, C], bf16, tag="wvt", name="wvt")
        dma(wvt[:, :, :], wvT.rearrange('(c p) m -> p c m', p=128))
        wot = wpool.tile([128, 4, C], bf16, tag="wot", name="wot")
        dma(wot[:, :, :], woT.rearrange('(c p) m -> p c m', p=128))
        wqk_t = [wqt[:, i, :] for i in range(NCT)]
        wv_t = [wvt[:, i, :] for i in range(NCT)]
        wo_t = [wot[:, i, :] for i in range(NCT)]

        # ---- P1: group stats -> per-channel scale s_col / shift t4 ----
        # per-ci rstd chains: ci0-2's stats land ~2us before ci3's, so
        # their sqrt -> reciprocal -> broadcast legs complete early and only
        # ci3's short chain stays on the critical path to the qk projection
        sqv = spool.tile([8, 4], f32, tag="sqv", name="sqv")
        pc = ps_z.tile([128, 8], f32, tag="zt", name="pc")
        for ci in range(NCT):
            nc.scalar.activation(sqv[:, ci:ci + 1], vneg[:, ci:ci + 1],
                                 AF.Sqrt, bias=epst[0:8, :], scale=-1.0)
            with nc.allow_low_precision(reason="exact DVE divide"):
                nc.vector.reciprocal(sg[:, 2 * ci + 1:2 * ci + 2],
                                     sqv[:, ci:ci + 1])
            nc.tensor.matmul(pc[:, 2 * ci:2 * ci + 2], bm[:],
                             sg[:, 2 * ci:2 * ci + 2], start=True, stop=True)
        # wqk scaled, bf16 (fp8 here would square the score noise); the
        # scale comes straight from the PSUM broadcast
        wqk_s = [wpool.tile([128, C], bf16, tag=f"wqs{i}", name=f"wqs{i}")
                 for i in range(NCT)]
        for ci in range(NCT):
            nc.vector.tensor_scalar_mul(wqk_s[ci][:], wqk_t[ci],
                                        pc[:, 2 * ci + 1:2 * ci + 2])
        rsb4 = consts.tile([128, 4], f32, tag="rsb4", name="rsb4")
        nc.vector.tensor_copy(rsb4[:], pc[:, 1:8:2])
        # t4 = -gmean*rstd (the y-form shift; beta terms are host-folded)
        t4 = consts.tile([128, 4], bf16, tag="t4", name="t4")
        nc.vector.scalar_tensor_tensor(out=t4[:], in0=pc[:, 0:8:2],
                                       scalar=-1.0,
                                       in1=rsb4[:], op0=MUL, op1=MUL)
        s64_4 = consts.tile([128, 4], f32, tag="s64_4", name="s64_4")
        nc.vector.scalar_tensor_tensor(out=s64_4[:], in0=rsb4[:], scalar=QKS,
                                       in1=gam4, op0=MUL, op1=MUL)
        s16_4 = consts.tile([128, 4], f32, tag="s16_4", name="s16_4")
        nc.vector.tensor_scalar_mul(s16_4[:], rsb4[:], 16.0)

        # ---- qk projection per i-block (bf16) + fp8 cast; the mt-sliced
        # PSUM lets casts chase the accumulation ----
        qk8 = [[qkp.tile([128, 2, 512], f8, tag=f"qk8_{s}_{ih}",
                         name=f"qk8_{s}_{ih}") for ih in range(2)]
               for s in range(2)]
        us4 = consts.tile([128, 4], f32, tag="us4", name="us4")
        su64_4 = consts.tile([128, 4], f32, tag="su64_4", name="su64_4")
        bvt4 = consts.tile([128, 4], bf16, tag="bvt4", name="bvt4")
        bos4 = consts.tile([128, 4], f32, tag="bos4", name="bos4")

        def gen_qk(ic, folds=False):
            qk_ps = [ps_big.tile([128, 2, 256], f32, tag="big",
                                 name=f"qkps{ic}_{half}") for half in range(2)]
            def cast_mt(mt):
                # ic0 casts on ACT (idle during the prologue; DVE still owns
                # the stats chain); later ics on DVE (truncating, but the
                # error headroom covers it) keeping ACT clear for exp
                dst = qk8[mt // 2][ic // 2][:, mt % 2,
                                            (ic % 2) * 256:(ic % 2) * 256 + 256]
                if ic == 0 and mt < 2:
                    nc.scalar.activation(
                        dst, qk_ps[mt // 2][:, mt % 2, :], AF.Identity,
                        scale=s64_4[:, mt:mt + 1], bias=su64_4[:, mt:mt + 1])
                else:
                    nc.vector.tensor_scalar(
                        dst, qk_ps[mt // 2][:, mt % 2, :],
                        s64_4[:, mt:mt + 1], su64_4[:, mt:mt + 1],
                        op0=MUL, op1=ADD)

            for mt in range(NCT):
                qkps = qk_ps[mt // 2][:, mt % 2, :]
                m_sl = slice(mt * 128, (mt + 1) * 128)
                for ct in range(NCT):
                    nc.tensor.matmul(qkps, wqk_s[ct][:, m_sl],
                                     x_t[ct][0][:, ic * 256:ic * 256 + 256],
                                     start=(ct == 0), stop=(ct == NCT - 1))
                if folds and mt == 0:
                    # P2 bias folds ride the PE stream right after the first
                    # qk tile: u = wqk^T t + hq (per out-channel), then
                    # bvt = wv^T t + bv. su64 = (u) * 64*scol gates the casts.
                    pq = ps_z.tile([128, 4], f32, tag="zt", name="pq")
                    for ot in range(NCT):
                        o_sl = slice(ot * 128, (ot + 1) * 128)
                        for ci in range(NCT):
                            nc.tensor.matmul(pq[:, ot:ot + 1],
                                             wqk_t[ci][:, o_sl],
                                             t4[:, ci:ci + 1],
                                             start=(ci == 0),
                                             stop=(ci == NCT - 1))
                    nc.vector.tensor_add(us4[:], pq[:, :],
                                         cp[:, 16:20].bitcast(f32))
                    nc.vector.tensor_mul(su64_4[:], us4[:], s64_4[:])
                    pv = ps_z.tile([128, 4], f32, tag="zt", name="pv")
                    for ot in range(NCT):
                        o_sl = slice(ot * 128, (ot + 1) * 128)
                        for ci in range(NCT):
                            nc.tensor.matmul(pv[:, ot:ot + 1],
                                             wv_t[ci][:, o_sl],
                                             t4[:, ci:ci + 1],
                                             start=(ci == 0),
                                             stop=(ci == NCT - 1))
                    bvf = spool.tile([128, 4], f32, tag="bvf", name="bvf")
                    nc.vector.tensor_add(bvf[:], pv[:, :],
                                         cp[:, 20:24].bitcast(f32))
                    nc.vector.tensor_copy(bvt4[:], bvf[:])
                yield
            for mt in range(NCT):
                cast_mt(mt)
                yield

        # wv -> fp8 packed with GN scale (x16 against subnormals) folded;
        # DVE so neither Pool (x-pack) nor ACT (casts+exp) stalls V
        wv8 = [wpool.tile([128, 2, C], f8, tag=f"wv8_{s}", name=f"wv8_{s}")
               for s in range(2)]

        def emit_wv8():
            for ci in range(NCT):
                if ci < 2:
                    nc.scalar.activation(wv8[ci // 2][:, ci % 2, :],
                                         wv_t[ci], AF.Identity,
                                         scale=s16_4[:, ci:ci + 1])
                else:
                    nc.vector.tensor_scalar_mul(wv8[ci // 2][:, ci % 2, :],
                                                wv_t[ci], s16_4[:, ci:ci + 1])

        # wo -> fp8 packed (raw weights; output projection runs DR)
        wo8 = [wpool.tile([128, 2, C], f8, tag=f"wo8_{s}", name=f"wo8_{s}")
               for s in range(2)]

        def emit_wo8():
            for ci in range(NCT):
                nc.gpsimd.tensor_copy(wo8[ci // 2][:, ci % 2, :], wo_t[ci])

        def emit_bo_fold():
            pb = ps_z.tile([128, 4], f32, tag="zt", name="pb")
            for ot in range(NCT):
                o_sl = slice(ot * 128, (ot + 1) * 128)
                for ci in range(NCT):
                    nc.tensor.matmul(pb[:, ot:ot + 1], wo_t[ci][:, o_sl],
                                     bvt4[:, ci:ci + 1],
                                     start=(ci == 0), stop=(ci == NCT - 1))
            nc.vector.tensor_add(bos4[:], pb[:, :], cp[:, 24:28].bitcast(f32))

        # ---- V projection (fp8 DR) on the 2-bank ps_v ring, one jt per
        # tile, interleaved into the scores streams; drains on DVE ----
        v8 = [vpool.tile([128, 2, C], f8, tag=f"v8_{p}", name=f"v8_{p}")
              for p in range(16)]

        def emit_vjt(jt):
            vt = ps_v.tile([128, 512], f32, tag="v", name=f"vt{jt}")
            for ch2 in range(2):
                for s in range(2):
                    nc.tensor.matmul(
                        vt[:, ch2 * 256:ch2 * 256 + 256],
                        xp[s][:, :, jt * 128:jt * 128 + 128],
                        wv8[s][:, :, ch2 * 256:ch2 * 256 + 256],
                        start=(s == 0), stop=(s == 1), perf_mode=DR)
            # drains on DVE: ACT is the exp pacer and must stay clear. DVE
            # truncates fp8 casts but V noise is fp8-quantization-dominated.
            nc.vector.tensor_copy(v8[jt // 2][:, jt % 2, :], vt[:, :])

        # ---- main attention loop over i-blocks of 256 ----
        pt8 = {}
        pending = []

        def drain(n):
            done = 0
            while pending and done < n:
                try:
                    next(pending[0])
                    done += 1
                except StopIteration:
                    pending.pop(0)

        def emit_scores(ic, with_v=(), per_quad=0):
            pt8[ic] = []
            vq = {q: [] for q in range(16)}
            for i, jt in enumerate(with_v):
                vq[min(15, 2 + i * 16 // len(with_v))].append(jt)
            for q in range(16):
                sq = ps_big.tile([128, 2, 256], f32, tag="big",
                                 name=f"sq{ic}_{q}")
                for jq in range(2):
                    jt = q * 2 + jq
                    for s in range(2):
                        nc.tensor.matmul(
                            sq[:, jq, :],
                            xp[s][:, :, jt * 128:jt * 128 + 128],
                            qk8[s][ic // 2][:, :, (ic % 2) * 256:
                                            (ic % 2) * 256 + 256],
                            start=(s == 0), stop=(s == 1), perf_mode=DR)
                pt = ptp.tile([128, 2, 256], f8, tag="pt", name=f"pt{ic}_{q}")
                nc.scalar.activation(pt[:, :, :], sq[:, :, :], AF.Exp,
                                     scale=1.0 / QKS, bias=ebias[:])
                pt8[ic].append(pt)
                for jt in vq[q]:
                    emit_vjt(jt)
                drain(per_quad)

        Rs = {}

        def gen_z(ic):
            """z chain -> R, yielded per quad for PE interleaving."""
            zt = ps_z.tile([128, 512], f32, tag="zt", name=f"zt{ic}")
            for q in range(16):
                while len(pt8.get(ic, ())) <= q:
                    yield   # scores for this quad not emitted yet
                nc.tensor.matmul(zt[:, 0:256], ones8[:, :, :],
                                 pt8[ic][q][:, :, :],
                                 start=(q == 0), stop=(q == 15), perf_mode=DR)
                if q % 2 == 1:
                    yield
            R = rpool.tile([128, 256], f32, tag="R", name=f"R{ic}")
            with nc.allow_low_precision(reason="exact DVE divide"):
                nc.vector.reciprocal(R[:], zt[:, 0:256])
            Rs[ic] = R

        def gen_ofin(ic):
            """O accumulation -> normalize -> output proj -> residual+store.
            Tiles rotate over ps_o / ps_v (free after V-projection) for ring
            depth; z keeps ps_z."""
            while ic not in Rs:
                yield   # z chain for this i-block not finished emitting
            R = Rs[ic]
            osb8 = [osbp.tile([128, 2, 256], f8, tag="osb",
                              name=f"osb{ic}_{s}") for s in range(2)]
            for mt in range(NCT):
                pool = ps_o if mt % 2 == 0 else ps_v
                ot_ps = pool.tile([128, 512], f32,
                                  tag="o" if mt % 2 == 0 else "v",
                                  name=f"o{ic}_{mt}")
                for p in range(16):
                    while len(pt8.get(ic, ())) <= p:
                        yield   # this quad's scores not emitted yet
                    nc.tensor.matmul(ot_ps[:, 0:256],
                                     v8[p][:, :, mt * 128:mt * 128 + 128],
                                     pt8[ic][p][:, :, :],
                                     start=(p == 0), stop=(p == 15),
                                     perf_mode=DR)
                    if p % 4 == 3:
                        yield
                nc.vector.tensor_mul(osb8[mt // 2][:, mt % 2, :],
                                     ot_ps[:, 0:256], R[:])
            for ot in range(NCT):
                o_sl = slice(ot * 128, (ot + 1) * 128)
                pool = ps_o if ot % 2 == 0 else ps_v
                ft = pool.tile([128, 512], f32,
                               tag="o" if ot % 2 == 0 else "v",
                               name=f"f{ic}_{ot}")
                for s in range(2):
                    nc.tensor.matmul(ft[:, 0:256], wo8[s][:, :, o_sl],
                                     osb8[s][:, :, :], start=(s == 0),
                                     stop=(s == 1), perf_mode=DR)
                ot_sb = outp.tile([128, 256], f32, tag="outsb",
                                  name=f"ot{ic}_{ot}")
                n0 = ic * 256
                h, off = n0 // 1024, n0 % 1024
                nc.vector.scalar_tensor_tensor(
                    out=ot_sb[:], in0=ft[:, 0:256], scalar=bos4[:, ot:ot + 1],
                    in1=x_t[ot][h][:, off:off + 256],
                    op0=ADD, op1=ADD)
                # spread final stores across queues: the tail DMAs otherwise
                # serialize behind one engine's issue overhead
                eng = [dma, dmaa, dma, dmaa][ot] if ic == 3 else dma
                eng(out[o_sl, ic * 256:ic * 256 + 256], ot_sb[:])
                yield

        # --- software-pipelined emission; the O/fin part of stream ic drains
        # one stream later than its z part so every v8 write is emitted
        # before any consumer ---
        for _ in gen_qk(0, folds=True):
            pass
        emit_wv8()
        emit_wo8()
        for _ in gen_qk(1):
            pass
        emit_scores(0, with_v=range(0, 10), per_quad=2)
        emit_bo_fold()
        pending.append(gen_z(0))
        pending.append(gen_qk(2))
        emit_scores(1, with_v=range(10, 32), per_quad=4)
        pending.append(gen_ofin(0))
        pending.append(gen_qk(3))
        pending.append(gen_z(1))
        pending.append(gen_ofin(1))
        emit_scores(2, per_quad=7)
        pending.append(gen_z(2))
        pending.append(gen_z(3))
        pending.append(gen_ofin(2))
        pending.append(gen_ofin(3))
        emit_scores(3, per_quad=8)
        drain(10 ** 9)

    _legalize_waits(nc, mybir)
    return nc


def kernel(**inputs):
    import ml_dtypes
    import concourse.bass  # noqa: F401
    from concourse.bass_utils import run_bass_kernel_spmd

    bft = ml_dtypes.bfloat16
    x = np.asarray(inputs["x"], dtype=np.float32)
    gamma = np.asarray(inputs["gamma"], np.float32)
    beta = np.asarray(inputs["beta"], np.float32)
    wq = np.asarray(inputs["wq"], np.float32)
    bq = np.asarray(inputs["bq"], np.float32)
    wk = np.asarray(inputs["wk"], np.float32)
    wv = np.asarray(inputs["wv"], np.float32)
    wo = np.asarray(inputs["wo"], np.float32)
    bv = np.asarray(inputs["bv"], np.float32)
    bo = np.asarray(inputs["bo"], np.float32)

    Bb, Cc, H, W = x.shape
    scale = Cc ** (-0.5)
    xf = x.reshape(Bb, Cc, H * W)

    wqk_raw = scale * (wq.T @ wk)
    hq = scale * (wk.T @ bq) + wqk_raw.T @ beta   # [C] (+ beta fold)
    wqk_h = np.ascontiguousarray(wqk_raw * gamma[:, None]).astype(bft)
    bvh = bv + wv @ beta                          # beta fold for V
    wvT = np.ascontiguousarray((wv * gamma[None, :]).T).astype(bft)
    woT = np.ascontiguousarray(wo.T).astype(bft)

    cpack = np.zeros((128, 44), np.float32)
    for p in range(128):
        cpack[p, p // 16] = 1.0 / 16.0            # gmask (x 1/16)
        cpack[p, 28 + p // 16] = 1.0 / (16.0 * 4096.0)   # raw-sum gmask
        cpack[p, 36 + p // 16] = 0.75 / 16.0      # 3/4-weighted gmask (ci1)
    cpack[:, 8:12] = gamma.reshape(NCT, 128).T
    cpack[:, 12:16] = beta.reshape(NCT, 128).T
    cpack[:, 16:20] = hq.reshape(NCT, 128).T
    cpack[:, 20:24] = bvh.reshape(NCT, 128).T
    cpack[:, 24:28] = bo.reshape(NCT, 128).T
    bmask = np.zeros((8, 128), np.float32)
    for p in range(128):
        bmask[p // 16, p] = 1.0

    if "nc" not in _cache:
        _cache["nc"] = _build()
    nc = _cache["nc"]

    in_maps = []
    for core in range(8):
        b, qi = core // 4, core % 4
        xb = xf[b]
        # rotate columns so this core's query quarter sits at columns [0, NQ)
        xrot = np.ascontiguousarray(
            np.concatenate([xb[:, qi * NQ:], xb[:, :qi * NQ]],
                           axis=1)).astype(bft)
        in_maps.append({
            "x": xrot, "wqk": wqk_h, "wvT": wvT, "woT": woT,
            "cpack": cpack, "bmask": bmask,
        })

    res = run_bass_kernel_spmd(nc, in_maps, core_ids=list(range(8)))
    outf = np.empty((Bb, Cc, H * W), np.float32)
    for core in range(8):
        b, qi = core // 4, core % 4
        outf[b][:, qi * NQ:(qi + 1) * NQ] = res.results[core]["out"]
    return outf.reshape(Bb, Cc, H, W)


# revision 104
# speedup vs baseline: 1.0056x; 1.0025x over previous
"""AttnBlock (GroupNorm -> single-head attention over H*W -> proj -> residual)
for Trainium2, 8 NeuronCores via SPMD -- bf16-ingest fp8 DoubleRow edition.

Sharding: core = b*4 + qi (b = batch 0/1, qi = query-quarter 0..3). The host
rotates each core's x columns so its query quarter is always columns [0, NQ)
(softmax reduces over keys, so key order is irrelevant).

x and the three weight matrices stream in as bf16 (halves the HBM prologue
that gates GroupNorm stats and therefore every matmul). All large matmuls
(scores, O=V*P, V projection, softmax denominator z, output projection) run
as fp8e4m3 DoubleRow (0.5 cyc/row); the qk projection runs bf16 (1 cyc/row)
because fp8 there squares the score noise. Softmax uses a global shift folded
into the Exp activation bias with a x16 premultiplier keeping P in fp8 normal
range; the x64 scaling of the fp8 qk operand is divided back out by the Exp
scale. GroupNorm scale folds into the bf16 wqk scaling (input side) and the
qk cast scale/bias + wv8 cast (output/keys side), so the fp8 x-pack is
stats-free and overlaps the x DMA.

Schedule: x tiles land h-major; bn_stats chunks chase the DMA on DVE and the
fp8 x-pack chases it on Pool (3 tiles on ACT). The stats -> rstd -> scol
chain resolves ~1.5us after the last tile; wqk lands right behind x, and the
per-i-block qk projection + scores stream starts immediately. V projection,
z (DoubleRow ones-matmul), O accumulation and the output projection pipeline
through the scores stream exactly as in the f32 edition, with V-tile drains
on DVE and the trailing per-i-block work drained into PE idle slots.
"""
import sys

sys.path.insert(0, '/opt/trn_rl_repo')

import numpy as np

C = 512
NG = 32
EPS = 1e-6
B = 2
N = 4096          # H*W
NQ = 1024         # query quarter per core
NCT = 4           # C // 128
EXPC = 5.5        # global softmax shift
QKS = 64.0        # fp8 qk pre-scale
EBIAS = -EXPC + float(np.log(16.0))   # exp bias: e^(s - 5.5 + ln16)

_cache = {}


def _legalize_waits(nc, mybir):
    """Codegen allows exactly ONE sync wait per instruction. Hoist excess
    waits onto preceding same-engine NoOps (semantics preserving)."""
    gen = 0
    for f in nc.m.functions:
        for bb in f.blocks:
            insts = list(bb.instructions)
            out = []
            changed = False
            for inst in insts:
                si = inst.sync_info
                waits = list(si.on_wait) if si and si.on_wait else []
                if len(waits) > 1:
                    for w in waits[:-1]:
                        gen += 1
                        nop = mybir.InstNoOp(
                            name=f"waitnop_{gen}", ins=[], outs=[],
                            engine=inst.engine)
                        nop.sync_info = mybir.SyncInfo(on_wait=[w], on_update=[])
                        out.append(nop)
                    inst.sync_info = mybir.SyncInfo(
                        on_wait=[waits[-1]],
                        on_update=list(si.on_update) if si and si.on_update else [])
                    changed = True
                out.append(inst)
            if changed:
                bb.instructions = out


def _build():
    import concourse.bass as bass
    import concourse.tile as tile
    from concourse import mybir
    from contextlib import ExitStack

    f32r = mybir.dt.float32r
    f32 = mybir.dt.float32
    bf16 = mybir.dt.bfloat16
    f8 = mybir.dt.float8e4
    AF = mybir.ActivationFunctionType
    DR = mybir.MatmulPerfMode.DoubleRow
    MUL = mybir.AluOpType.mult
    ADD = mybir.AluOpType.add

    nc = bass.Bass(trn_type="TRN2", target_bir_lowering=False, debug=False)

    x = nc.dram_tensor("x", [C, N], bf16, kind="ExternalInput").ap()
    wqk = nc.dram_tensor("wqk", [C, C], bf16, kind="ExternalInput").ap()
    wvT = nc.dram_tensor("wvT", [C, C], bf16, kind="ExternalInput").ap()
    woT = nc.dram_tensor("woT", [C, C], bf16, kind="ExternalInput").ap()
    cpack = nc.dram_tensor("cpack", [128, 44], f32, kind="ExternalInput").ap()
    bmask = nc.dram_tensor("bmask", [8, 128], f32, kind="ExternalInput").ap()
    out = nc.dram_tensor("out", [C, NQ], f32, kind="ExternalOutput").ap()

    dma = nc.sync.dma_start
    dmap = nc.gpsimd.dma_start
    dmaa = nc.scalar.dma_start

    with tile.TileContext(nc) as tc, ExitStack() as top:
        xpool = top.enter_context(tc.tile_pool(name="xpool", bufs=1))
        consts = top.enter_context(tc.tile_pool(name="consts", bufs=1))
        wpool = top.enter_context(tc.tile_pool(name="wpool", bufs=1))
        xq8p = top.enter_context(tc.tile_pool(name="xq8p", bufs=1))
        qkp = top.enter_context(tc.tile_pool(name="qkp", bufs=1))
        vpool = top.enter_context(tc.tile_pool(name="vpool", bufs=1))
        ptp = top.enter_context(tc.tile_pool(name="ptp", bufs=48))
        spool = top.enter_context(tc.tile_pool(name="spool", bufs=1))
        osbp = top.enter_context(tc.tile_pool(name="osbp", bufs=8))
        rpool = top.enter_context(tc.tile_pool(name="rpool", bufs=4))
        outp = top.enter_context(tc.tile_pool(name="outp", bufs=16))
        ps_big = top.enter_context(tc.tile_pool(name="ps_big", bufs=4, space="PSUM"))
        ps_v = top.enter_context(tc.tile_pool(name="ps_v", bufs=2, space="PSUM"))
        ps_o = top.enter_context(tc.tile_pool(name="ps_o", bufs=1, space="PSUM"))
        ps_z = top.enter_context(tc.tile_pool(name="ps_z", bufs=1, space="PSUM"))

        # ---- consts: one packed DMA + bmask (Pool SWDGE queue: keeps the
        # HWDGE pipe clear for the x stream) ----
        cp = consts.tile([128, 44], f32r, tag="cp", name="cp")
        dmap(cp[:], cpack[:, :].bitcast(f32r))
        bm = consts.tile([8, 128], f32r, tag="bm", name="bm")
        dmap(bm[:], bmask.bitcast(f32r))
        gm = cp[:, 0:8]
        gm0 = cp[:, 28:36]
        gm75 = cp[:, 36:44]
        gam4 = cp[:, 8:12].bitcast(f32)
        bet4 = cp[:, 12:16].bitcast(f32)
        hqc = [cp[:, 16 + i:17 + i].bitcast(f32) for i in range(NCT)]
        bvc = [cp[:, 20 + i:21 + i].bitcast(f32) for i in range(NCT)]
        boc = [cp[:, 24 + i:25 + i].bitcast(f32) for i in range(NCT)]

        epst = consts.tile([128, 1], f32, tag="epst", name="epst")
        nc.vector.memset(epst[:], EPS)
        ebias = consts.tile([128, 1], f32, tag="ebias", name="ebias")
        nc.vector.memset(ebias[:], EBIAS)
        # z-ones are 16.0: they exactly cancel the x16 pre-scale on wv8
        # (kept out of e4m3 subnormal range), since o/z is scale-invariant
        ones8 = consts.tile([128, 2, 128], f8, tag="ones8", name="ones8")
        nc.vector.memset(ones8[:, :, :], 16.0)

        # prime the PE p-state clock: one tiny DR matmul right at t~0 so the
        # >3us ramp has elapsed by the time the real matmul stream starts
        prm = ps_z.tile([128, 2], f32, tag="zt", name="prm")
        nc.tensor.matmul(prm[:], ones8[:, :, 0:128], ones8[:, :, 0:2],
                         start=True, stop=True, perf_mode=DR)

        # ---- x resident first (h-major so fp8-pack column blocks complete
        # early). Stats chase the DMA: DVE runs bn_stats for ci 1-3 while
        # ACT covers ci0 with a fused fp8-cast+sum pass plus a Square+sum
        # pass (its xp tiles ride along for free); Pool casts the rest.
        # The group-sum matmul accumulates per ci as aggregates land. ----
        x_t = [[xpool.tile([128, 1024], bf16, tag=f"x{ci}_{h}",
                           name=f"x{ci}_{h}") for h in range(4)]
               for ci in range(NCT)]
        xp = [xq8p.tile([128, 2, N], f8, tag=f"xp{s}", name=f"xp{s}")
              for s in range(2)]
        statsAll = spool.tile([128, 8], f32r, tag="stA", name="statsAll")
        mvt = [spool.tile([128, 2], f32, tag=f"mv{i}", name=f"mv{i}")
               for i in range(NCT)]
        stats3 = [spool.tile([128, 8, 6], f32, tag=f"st3{i}", name=f"st3{i}")
                  for i in range(1, NCT)]
        aS = spool.tile([128, 4, 2], f32r, tag="aS", name="aS")
        aB = spool.tile([128, 2, 2], f32r, tag="aB", name="aB")
        sqscr = spool.tile([128, 1024], bf16, tag="sqscr", name="sqscr")
        # one PSUM accumulation region per ci: overlapping accumulation
        # groups in one bank are illegal, and the four group-sum streams
        # overlap in time. ps_o / ps_v are idle during the prologue.
        ssums = [ps_o.tile([8, 2], f32, tag="o", name="ss0"),
                 ps_v.tile([8, 2], f32, tag="v", name="ss1"),
                 ps_v.tile([8, 2], f32, tag="v", name="ss2"),
                 ps_z.tile([8, 2], f32, tag="zt", name="ss3")]
        sg = spool.tile([8, 8], f32r, tag="sg", name="sg")
        vneg = spool.tile([8, 4], f32, tag="vneg", name="vneg")
        for h in range(4):
            for ci in (0, 1, 2, 3):
                dma(x_t[ci][h][:],
                    x[ci * 128:(ci + 1) * 128, h * 1024:(h + 1) * 1024])
                dst = xp[ci // 2][:, ci % 2, h * 1024:(h + 1) * 1024]
                if ci == 0:
                    with nc.allow_low_precision(reason="f32r group sums"):
                        nc.scalar.activation(dst, x_t[ci][h][:], AF.Copy,
                                             accum_out=aS[:, h, 0:1])
                        nc.scalar.activation(sqscr[:], x_t[ci][h][:],
                                             AF.Square,
                                             accum_out=aS[:, h, 1:2])
                    # raw-sum group aggregation rides the PE with a mask
                    # pre-scaled by 1/(16*4096); no DVE combine ops at all
                    nc.tensor.matmul(ssums[0][:, :], gm0, aS[:, h, :],
                                     start=(h == 0), stop=(h == 3))
                    if h == 3:
                        nc.vector.tensor_copy(sg[:, 0:2], ssums[0][:, :])
                        nc.vector.scalar_tensor_tensor(
                            out=vneg[:, 0:1], in0=sg[:, 0:1],
                            scalar=sg[:, 0:1], in1=sg[:, 1:2],
                            op0=MUL, op1=mybir.AluOpType.subtract)
                    continue
                if ci in (1, 2) and h == 0:
                    # the h0 tiles of ci1/ci2 ride ACT too, easing the DVE
                    # bn_stats backlog; their raw sums fold into ssum via gm0
                    # while the bn path for h1-3 gets a 3/4-scaled mask
                    with nc.allow_low_precision(reason="f32r group sums"):
                        nc.scalar.activation(dst, x_t[ci][h][:], AF.Copy,
                                             accum_out=aB[:, ci - 1, 0:1])
                        nc.scalar.activation(sqscr[:], x_t[ci][h][:],
                                             AF.Square,
                                             accum_out=aB[:, ci - 1, 1:2])
                    nc.tensor.matmul(ssums[ci][:, :], gm0,
                                     aB[:, ci - 1, :],
                                     start=True, stop=False)
                    continue
                for k in range(2):
                    nc.vector.bn_stats(
                        out=stats3[ci - 1][:, h * 2 + k, :],
                        in_=x_t[ci][h][:, k * 512:(k + 1) * 512])
                nc.gpsimd.tensor_copy(dst, x_t[ci][h][:])
                if h == 3:
                    mv = mvt[ci]
                    in3 = (stats3[ci - 1][:, 2:8, :] if ci in (1, 2)
                           else stats3[ci - 1][:, :, :])
                    nc.vector.bn_aggr(out=mv[:], in_=in3)
                    nc.vector.tensor_copy(statsAll[:, 2 * ci:2 * ci + 1],
                                          mv[:, 0:1])
                    # E[x^2] = mean^2 + var in one fused op
                    nc.vector.scalar_tensor_tensor(
                        out=statsAll[:, 2 * ci + 1:2 * ci + 2], in0=mv[:, 0:1],
                        scalar=mv[:, 0:1], in1=mv[:, 1:2], op0=MUL, op1=ADD)
                    nc.tensor.matmul(ssums[ci][:, :],
                                     gm75 if ci in (1, 2) else gm,
                                     statsAll[:, 2 * ci:2 * ci + 2],
                                     start=(ci not in (1, 2)), stop=True)
                    nc.vector.tensor_copy(sg[:, 2 * ci:2 * ci + 2],
                                          ssums[ci][:, :])
                    # -var = mean^2 - E[x^2], fused per ci as its slice lands
                    nc.vector.scalar_tensor_tensor(
                        out=vneg[:, ci:ci + 1], in0=sg[:, 2 * ci:2 * ci + 1],
                        scalar=sg[:, 2 * ci:2 * ci + 1],
                        in1=sg[:, 2 * ci + 1:2 * ci + 2],
                        op0=MUL, op1=mybir.AluOpType.subtract)

        # ---- weights (land right after x on the HBM queue; single DMA
        # each via a strided dram view) ----
        wqt = wpool.tile([128, 4, C], bf16, tag="wqt", name="wqt")
        dma(wqt[:, :, :], wqk.rearrange('(c p) m -> p c m', p=128))
        wvt = wpool.tile([128, 4, C], bf16, tag="wvt", name="wvt")
        dma(wvt[:, :, :], wvT.rearrange('(c p) m -> p c m', p=128))
        wot = wpool.tile([128, 4, C], bf16, tag="wot", name="wot")
        dma(wot[:, :, :], woT.rearrange('(c p) m -> p c m', p=128))
        wqk_t = [wqt[:, i, :] for i in range(NCT)]
        wv_t = [wvt[:, i, :] for i in range(NCT)]
        wo_t = [wot[:, i, :] for i in range(NCT)]

        # ---- P1: group stats -> per-channel scale s_col / shift t4 ----
        # per-ci rstd chains: ci0-2's stats land ~2us before ci3's, so
        # their sqrt -> reciprocal -> broadcast legs complete early and only
        # ci3's short chain stays on the critical path to the qk projection
        # reciprocal first (stays on DVE, back-to-back with the fused
        # variance) then sqrt(-1/x) on ACT: one fewer cross-engine hop per ci
        vne = spool.tile([8, 4], f32, tag="vne", name="vne")
        rcv = spool.tile([8, 4], f32, tag="rcv", name="rcv")
        pc = ps_z.tile([128, 8], f32, tag="zt", name="pc")
        for ci in range(NCT):
            nc.vector.tensor_scalar_add(vne[:, ci:ci + 1],
                                        vneg[:, ci:ci + 1], -EPS)
            with nc.allow_low_precision(reason="exact DVE divide"):
                nc.vector.reciprocal(rcv[:, ci:ci + 1], vne[:, ci:ci + 1])
                nc.scalar.activation(sg[:, 2 * ci + 1:2 * ci + 2],
                                     rcv[:, ci:ci + 1], AF.Sqrt, scale=-1.0)
            nc.tensor.matmul(pc[:, 2 * ci:2 * ci + 2], bm[:],
                             sg[:, 2 * ci:2 * ci + 2], start=True, stop=True)
        # wqk scaled, bf16 (fp8 here would square the score noise); the
        # scale comes straight from the PSUM broadcast
        wqk_s = [wpool.tile([128, C], bf16, tag=f"wqs{i}", name=f"wqs{i}")
                 for i in range(NCT)]
        for ci in range(NCT):
            nc.vector.tensor_scalar_mul(wqk_s[ci][:], wqk_t[ci],
                                        pc[:, 2 * ci + 1:2 * ci + 2])
        rsb4 = consts.tile([128, 4], f32, tag="rsb4", name="rsb4")
        nc.vector.tensor_copy(rsb4[:], pc[:, 1:8:2])
        # t4 = -gmean*rstd (the y-form shift; beta terms are host-folded)
        t4 = consts.tile([128, 4], bf16, tag="t4", name="t4")
        nc.vector.scalar_tensor_tensor(out=t4[:], in0=pc[:, 0:8:2],
                                       scalar=-1.0,
                                       in1=rsb4[:], op0=MUL, op1=MUL)
        s64_4 = consts.tile([128, 4], f32, tag="s64_4", name="s64_4")
        nc.vector.scalar_tensor_tensor(out=s64_4[:], in0=rsb4[:], scalar=QKS,
                                       in1=gam4, op0=MUL, op1=MUL)
        s16_4 = consts.tile([128, 4], f32, tag="s16_4", name="s16_4")
        nc.vector.tensor_scalar_mul(s16_4[:], rsb4[:], 16.0)

        # ---- qk projection per i-block (bf16) + fp8 cast; the mt-sliced
        # PSUM lets casts chase the accumulation ----
        qk8 = [[qkp.tile([128, 2, 512], f8, tag=f"qk8_{s}_{ih}",
                         name=f"qk8_{s}_{ih}") for ih in range(2)]
               for s in range(2)]
        us4 = consts.tile([128, 4], f32, tag="us4", name="us4")
        su64_4 = consts.tile([128, 4], f32, tag="su64_4", name="su64_4")
        bvt4 = consts.tile([128, 4], bf16, tag="bvt4", name="bvt4")
        bos4 = consts.tile([128, 4], f32, tag="bos4", name="bos4")

        def gen_qk(ic, folds=False):
            qk_ps = [ps_big.tile([128, 2, 256], f32, tag="big",
                                 name=f"qkps{ic}_{half}") for half in range(2)]
            def cast_mt(mt):
                # ic0 casts on ACT (idle during the prologue; DVE still owns
                # the stats chain); later ics on DVE (truncating, but the
                # error headroom covers it) keeping ACT clear for exp
                dst = qk8[mt // 2][ic // 2][:, mt % 2,
                                            (ic % 2) * 256:(ic % 2) * 256 + 256]
                if ic == 0 and mt < 2:
                    nc.scalar.activation(
                        dst, qk_ps[mt // 2][:, mt % 2, :], AF.Identity,
                        scale=s64_4[:, mt:mt + 1], bias=su64_4[:, mt:mt + 1])
                else:
                    nc.vector.tensor_scalar(
                        dst, qk_ps[mt // 2][:, mt % 2, :],
                        s64_4[:, mt:mt + 1], su64_4[:, mt:mt + 1],
                        op0=MUL, op1=ADD)

            for mt in range(NCT):
                qkps = qk_ps[mt // 2][:, mt % 2, :]
                m_sl = slice(mt * 128, (mt + 1) * 128)
                for ct in range(NCT):
                    nc.tensor.matmul(qkps, wqk_s[ct][:, m_sl],
                                     x_t[ct][0][:, ic * 256:ic * 256 + 256],
                                     start=(ct == 0), stop=(ct == NCT - 1))
                if folds and mt == 0:
                    # P2 bias folds ride the PE stream right after the first
                    # qk tile: u = wqk^T t + hq (per out-channel), then
                    # bvt = wv^T t + bv. su64 = (u) * 64*scol gates the casts.
                    pq = ps_z.tile([128, 4], f32, tag="zt", name="pq")
                    for ot in range(NCT):
                        o_sl = slice(ot * 128, (ot + 1) * 128)
                        for ci in range(NCT):
                            nc.tensor.matmul(pq[:, ot:ot + 1],
                                             wqk_t[ci][:, o_sl],
                                             t4[:, ci:ci + 1],
                                             start=(ci == 0),
                                             stop=(ci == NCT - 1))
                    nc.vector.tensor_add(us4[:], pq[:, :],
                                         cp[:, 16:20].bitcast(f32))
                    nc.vector.tensor_mul(su64_4[:], us4[:], s64_4[:])
                    pv = ps_z.tile([128, 4], f32, tag="zt", name="pv")
                    for ot in range(NCT):
                        o_sl = slice(ot * 128, (ot + 1) * 128)
                        for ci in range(NCT):
                            nc.tensor.matmul(pv[:, ot:ot + 1],
                                             wv_t[ci][:, o_sl],
                                             t4[:, ci:ci + 1],
                                             start=(ci == 0),
                                             stop=(ci == NCT - 1))
                    bvf = spool.tile([128, 4], f32, tag="bvf", name="bvf")
                    nc.vector.tensor_add(bvf[:], pv[:, :],
                                         cp[:, 20:24].bitcast(f32))
                    nc.vector.tensor_copy(bvt4[:], bvf[:])
                yield
            for mt in range(NCT):
                cast_mt(mt)
                yield

        # wv -> fp8 packed with GN scale (x16 against subnormals) folded;
        # DVE so neither Pool (x-pack) nor ACT (casts+exp) stalls V
        wv8 = [wpool.tile([128, 2, C], f8, tag=f"wv8_{s}", name=f"wv8_{s}")
               for s in range(2)]

        def emit_wv8():
            for ci in range(NCT):
                if ci < 2:
                    nc.scalar.activation(wv8[ci // 2][:, ci % 2, :],
                                         wv_t[ci], AF.Identity,
                                         scale=s16_4[:, ci:ci + 1])
                else:
                    nc.vector.tensor_scalar_mul(wv8[ci // 2][:, ci % 2, :],
                                                wv_t[ci], s16_4[:, ci:ci + 1])

        # wo -> fp8 packed (raw weights; output projection runs DR)
        wo8 = [wpool.tile([128, 2, C], f8, tag=f"wo8_{s}", name=f"wo8_{s}")
               for s in range(2)]

        def emit_wo8():
            for ci in range(NCT):
                nc.gpsimd.tensor_copy(wo8[ci // 2][:, ci % 2, :], wo_t[ci])

        def emit_bo_fold():
            pb = ps_z.tile([128, 4], f32, tag="zt", name="pb")
            for ot in range(NCT):
                o_sl = slice(ot * 128, (ot + 1) * 128)
                for ci in range(NCT):
                    nc.tensor.matmul(pb[:, ot:ot + 1], wo_t[ci][:, o_sl],
                                     bvt4[:, ci:ci + 1],
                                     start=(ci == 0), stop=(ci == NCT - 1))
            nc.vector.tensor_add(bos4[:], pb[:, :], cp[:, 24:28].bitcast(f32))

        # ---- V projection (fp8 DR) on the 2-bank ps_v ring, one jt per
        # tile, interleaved into the scores streams; drains on DVE ----
        v8 = [vpool.tile([128, 2, C], f8, tag=f"v8_{p}", name=f"v8_{p}")
              for p in range(16)]

        def emit_vjt(jt):
            vt = ps_v.tile([128, 512], f32, tag="v", name=f"vt{jt}")
            for ch2 in range(2):
                for s in range(2):
                    nc.tensor.matmul(
                        vt[:, ch2 * 256:ch2 * 256 + 256],
                        xp[s][:, :, jt * 128:jt * 128 + 128],
                        wv8[s][:, :, ch2 * 256:ch2 * 256 + 256],
                        start=(s == 0), stop=(s == 1), perf_mode=DR)
            # drains on DVE: ACT is the exp pacer and must stay clear. DVE
            # truncates fp8 casts but V noise is fp8-quantization-dominated.
            nc.vector.tensor_copy(v8[jt // 2][:, jt % 2, :], vt[:, :])

        # ---- main attention loop over i-blocks of 256 ----
        pt8 = {}
        pending = []

        def drain(n):
            done = 0
            while pending and done < n:
                try:
                    next(pending[0])
                    done += 1
                except StopIteration:
                    pending.pop(0)

        def emit_scores(ic, with_v=(), per_quad=0):
            pt8[ic] = []
            vq = {q: [] for q in range(16)}
            for i, jt in enumerate(with_v):
                vq[min(15, 2 + i * 16 // len(with_v))].append(jt)
            for q in range(16):
                sq = ps_big.tile([128, 2, 256], f32, tag="big",
                                 name=f"sq{ic}_{q}")
                for jq in range(2):
                    jt = q * 2 + jq
                    for s in range(2):
                        nc.tensor.matmul(
                            sq[:, jq, :],
                            xp[s][:, :, jt * 128:jt * 128 + 128],
                            qk8[s][ic // 2][:, :, (ic % 2) * 256:
                                            (ic % 2) * 256 + 256],
                            start=(s == 0), stop=(s == 1), perf_mode=DR)
                pt = ptp.tile([128, 2, 256], f8, tag="pt", name=f"pt{ic}_{q}")
                nc.scalar.activation(pt[:, :, :], sq[:, :, :], AF.Exp,
                                     scale=1.0 / QKS, bias=ebias[:])
                pt8[ic].append(pt)
                for jt in vq[q]:
                    emit_vjt(jt)
                drain(per_quad)

        Rs = {}

        def gen_z(ic):
            """z chain -> R, yielded per quad for PE interleaving."""
            zt = ps_z.tile([128, 512], f32, tag="zt", name=f"zt{ic}")
            for q in range(16):
                while len(pt8.get(ic, ())) <= q:
                    yield   # scores for this quad not emitted yet
                nc.tensor.matmul(zt[:, 0:256], ones8[:, :, :],
                                 pt8[ic][q][:, :, :],
                                 start=(q == 0), stop=(q == 15), perf_mode=DR)
                if q % 2 == 1:
                    yield
            R = rpool.tile([128, 256], f32, tag="R", name=f"R{ic}")
            with nc.allow_low_precision(reason="exact DVE divide"):
                nc.vector.reciprocal(R[:], zt[:, 0:256])
            Rs[ic] = R

        def gen_ofin(ic):
            """O accumulation -> normalize -> output proj -> residual+store.
            Tiles rotate over ps_o / ps_v (free after V-projection) for ring
            depth; z keeps ps_z."""
            while ic not in Rs:
                yield   # z chain for this i-block not finished emitting
            R = Rs[ic]
            osb8 = [osbp.tile([128, 2, 256], f8, tag="osb",
                              name=f"osb{ic}_{s}") for s in range(2)]
            for mt in range(NCT):
                pool = ps_o if mt % 2 == 0 else ps_v
                ot_ps = pool.tile([128, 512], f32,
                                  tag="o" if mt % 2 == 0 else "v",
                                  name=f"o{ic}_{mt}")
                for p in range(16):
                    while len(pt8.get(ic, ())) <= p:
                        yield   # this quad's scores not emitted yet
                    nc.tensor.matmul(ot_ps[:, 0:256],
                                     v8[p][:, :, mt * 128:mt * 128 + 128],
                                     pt8[ic][p][:, :, :],
                                     start=(p == 0), stop=(p == 15),
                                     perf_mode=DR)
                    if p % 4 == 3:
                        yield
                nc.vector.tensor_mul(osb8[mt // 2][:, mt % 2, :],
                                     ot_ps[:, 0:256], R[:])
            for ot in range(NCT):
                o_sl = slice(ot * 128, (ot + 1) * 128)
                pool = ps_o if ot % 2 == 0 else ps_v
                ft = pool.tile([128, 512], f32,
                               tag="o" if ot % 2 == 0 else "v",
                               name=f"f{ic}_{ot}")
                for s in range(2):
                    nc.tensor.matmul(ft[:, 0:256], wo8[s][:, :, o_sl],
                                     osb8[s][:, :, :], start=(s == 0),
                                     stop=(s == 1), perf_mode=DR)
                ot_sb = outp.tile([128, 256], f32, tag="outsb",
                                  name=f"ot{ic}_{ot}")
                n0 = ic * 256
                h, off = n0 // 1024, n0 % 1024
                nc.vector.scalar_tensor_tensor(
                    out=ot_sb[:], in0=ft[:, 0:256], scalar=bos4[:, ot:ot + 1],
                    in1=x_t[ot][h][:, off:off + 256],
                    op0=ADD, op1=ADD)
                # spread final stores across queues: the tail DMAs otherwise
                # serialize behind one engine's issue overhead
                eng = [dma, dmaa, dma, dmaa][ot] if ic == 3 else dma
                eng(out[o_sl, ic * 256:ic * 256 + 256], ot_sb[:])
                yield

        # --- software-pipelined emission; the O/fin part of stream ic drains
        # one stream later than its z part so every v8 write is emitted
        # before any consumer ---
        for _ in gen_qk(0, folds=True):
            pass
        emit_wv8()
        emit_wo8()
        for _ in gen_qk(1):
            pass
        emit_scores(0, with_v=range(0, 10), per_quad=2)
        emit_bo_fold()
        pending.append(gen_z(0))
        pending.append(gen_qk(2))
        emit_scores(1, with_v=range(10, 32), per_quad=4)
        pending.append(gen_ofin(0))
        pending.append(gen_qk(3))
        pending.append(gen_z(1))
        pending.append(gen_ofin(1))
        emit_scores(2, per_quad=7)
        pending.append(gen_z(2))
        pending.append(gen_z(3))
        pending.append(gen_ofin(2))
        pending.append(gen_ofin(3))
        emit_scores(3, per_quad=8)
        drain(10 ** 9)

    _legalize_waits(nc, mybir)
    return nc


def kernel(**inputs):
    import ml_dtypes
    import concourse.bass  # noqa: F401
    from concourse.bass_utils import run_bass_kernel_spmd

    bft = ml_dtypes.bfloat16
    x = np.asarray(inputs["x"], dtype=np.float32)
    gamma = np.asarray(inputs["gamma"], np.float32)
    beta = np.asarray(inputs["beta"], np.float32)
    wq = np.asarray(inputs["wq"], np.float32)
    bq = np.asarray(inputs["bq"], np.float32)
    wk = np.asarray(inputs["wk"], np.float32)
    wv = np.asarray(inputs["wv"], np.float32)
    wo = np.asarray(inputs["wo"], np.float32)
    bv = np.asarray(inputs["bv"], np.float32)
    bo = np.asarray(inputs["bo"], np.float32)

    Bb, Cc, H, W = x.shape
    scale = Cc ** (-0.5)
    xf = x.reshape(Bb, Cc, H * W)

    wqk_raw = scale * (wq.T @ wk)
    hq = scale * (wk.T @ bq) + wqk_raw.T @ beta   # [C] (+ beta fold)
    wqk_h = np.ascontiguousarray(wqk_raw * gamma[:, None]).astype(bft)
    bvh = bv + wv @ beta                          # beta fold for V
    wvT = np.ascontiguousarray((wv * gamma[None, :]).T).astype(bft)
    woT = np.ascontiguousarray(wo.T).astype(bft)

    cpack = np.zeros((128, 44), np.float32)
    for p in range(128):
        cpack[p, p // 16] = 1.0 / 16.0            # gmask (x 1/16)
        cpack[p, 28 + p // 16] = 1.0 / (16.0 * 4096.0)   # raw-sum gmask
        cpack[p, 36 + p // 16] = 0.75 / 16.0      # 3/4-weighted gmask (ci1)
    cpack[:, 8:12] = gamma.reshape(NCT, 128).T
    cpack[:, 12:16] = beta.reshape(NCT, 128).T
    cpack[:, 16:20] = hq.reshape(NCT, 128).T
    cpack[:, 20:24] = bvh.reshape(NCT, 128).T
    cpack[:, 24:28] = bo.reshape(NCT, 128).T
    bmask = np.zeros((8, 128), np.float32)
    for p in range(128):
        bmask[p // 16, p] = 1.0

    if "nc" not in _cache:
        _cache["nc"] = _build()
    nc = _cache["nc"]

    in_maps = []
    for core in range(8):
        b, qi = core // 4, core % 4
        xb = xf[b]
        # rotate columns so this core's query quarter sits at columns [0, NQ)
        xrot = np.ascontiguousarray(
            np.concatenate([xb[:, qi * NQ:], xb[:, :qi * NQ]],
                           axis=1)).astype(bft)
        in_maps.append({
            "x": xrot, "wqk": wqk_h, "wvT": wvT, "woT": woT,
            "cpack": cpack, "bmask": bmask,
        })

    res = run_bass_kernel_spmd(nc, in_maps, core_ids=list(range(8)))
    outf = np.empty((Bb, Cc, H * W), np.float32)
    for core in range(8):
        b, qi = core // 4, core % 4
        outf[b][:, qi * NQ:(qi + 1) * NQ] = res.results[core]["out"]
    return outf.reshape(Bb, Cc, H, W)


# revision 105
# speedup vs baseline: 1.0060x; 1.0004x over previous
"""AttnBlock (GroupNorm -> single-head attention over H*W -> proj -> residual)
for Trainium2, 8 NeuronCores via SPMD -- bf16-ingest fp8 DoubleRow edition.

Sharding: core = b*4 + qi (b = batch 0/1, qi = query-quarter 0..3). The host
rotates each core's x columns so its query quarter is always columns [0, NQ)
(softmax reduces over keys, so key order is irrelevant).

x and the three weight matrices stream in as bf16 (halves the HBM prologue
that gates GroupNorm stats and therefore every matmul). All large matmuls
(scores, O=V*P, V projection, softmax denominator z, output projection) run
as fp8e4m3 DoubleRow (0.5 cyc/row); the qk projection runs bf16 (1 cyc/row)
because fp8 there squares the score noise. Softmax uses a global shift folded
into the Exp activation bias with a x16 premultiplier keeping P in fp8 normal
range; the x64 scaling of the fp8 qk operand is divided back out by the Exp
scale. GroupNorm scale folds into the bf16 wqk scaling (input side) and the
qk cast scale/bias + wv8 cast (output/keys side), so the fp8 x-pack is
stats-free and overlaps the x DMA.

Schedule: x tiles land h-major; bn_stats chunks chase the DMA on DVE and the
fp8 x-pack chases it on Pool (3 tiles on ACT). The stats -> rstd -> scol
chain resolves ~1.5us after the last tile; wqk lands right behind x, and the
per-i-block qk projection + scores stream starts immediately. V projection,
z (DoubleRow ones-matmul), O accumulation and the output projection pipeline
through the scores stream exactly as in the f32 edition, with V-tile drains
on DVE and the trailing per-i-block work drained into PE idle slots.
"""
import sys

sys.path.insert(0, '/opt/trn_rl_repo')

import numpy as np

C = 512
NG = 32
EPS = 1e-6
B = 2
N = 4096          # H*W
NQ = 1024         # query quarter per core
NCT = 4           # C // 128
EXPC = 5.5        # global softmax shift
QKS = 64.0        # fp8 qk pre-scale
EBIAS = -EXPC + float(np.log(16.0))   # exp bias: e^(s - 5.5 + ln16)

_cache = {}


def _legalize_waits(nc, mybir):
    """Codegen allows exactly ONE sync wait per instruction. Hoist excess
    waits onto preceding same-engine NoOps (semantics preserving)."""
    gen = 0
    for f in nc.m.functions:
        for bb in f.blocks:
            insts = list(bb.instructions)
            out = []
            changed = False
            for inst in insts:
                si = inst.sync_info
                waits = list(si.on_wait) if si and si.on_wait else []
                if len(waits) > 1:
                    for w in waits[:-1]:
                        gen += 1
                        nop = mybir.InstNoOp(
                            name=f"waitnop_{gen}", ins=[], outs=[],
                            engine=inst.engine)
                        nop.sync_info = mybir.SyncInfo(on_wait=[w], on_update=[])
                        out.append(nop)
                    inst.sync_info = mybir.SyncInfo(
                        on_wait=[waits[-1]],
                        on_update=list(si.on_update) if si and si.on_update else [])
                    changed = True
                out.append(inst)
            if changed:
                bb.instructions = out


def _build():
    import concourse.bass as bass
    import concourse.tile as tile
    from concourse import mybir
    from contextlib import ExitStack

    f32r = mybir.dt.float32r
    f32 = mybir.dt.float32
    bf16 = mybir.dt.bfloat16
    f8 = mybir.dt.float8e4
    AF = mybir.ActivationFunctionType
    DR = mybir.MatmulPerfMode.DoubleRow
    MUL = mybir.AluOpType.mult
    ADD = mybir.AluOpType.add

    nc = bass.Bass(trn_type="TRN2", target_bir_lowering=False, debug=False)

    x = nc.dram_tensor("x", [C, N], bf16, kind="ExternalInput").ap()
    wqk = nc.dram_tensor("wqk", [C, C], bf16, kind="ExternalInput").ap()
    wvT = nc.dram_tensor("wvT", [C, C], bf16, kind="ExternalInput").ap()
    woT = nc.dram_tensor("woT", [C, C], bf16, kind="ExternalInput").ap()
    cpack = nc.dram_tensor("cpack", [128, 44], f32, kind="ExternalInput").ap()
    bmask = nc.dram_tensor("bmask", [8, 128], f32, kind="ExternalInput").ap()
    out = nc.dram_tensor("out", [C, NQ], f32, kind="ExternalOutput").ap()

    dma = nc.sync.dma_start
    dmap = nc.gpsimd.dma_start
    dmaa = nc.scalar.dma_start

    with tile.TileContext(nc) as tc, ExitStack() as top:
        xpool = top.enter_context(tc.tile_pool(name="xpool", bufs=1))
        consts = top.enter_context(tc.tile_pool(name="consts", bufs=1))
        wpool = top.enter_context(tc.tile_pool(name="wpool", bufs=1))
        xq8p = top.enter_context(tc.tile_pool(name="xq8p", bufs=1))
        qkp = top.enter_context(tc.tile_pool(name="qkp", bufs=1))
        vpool = top.enter_context(tc.tile_pool(name="vpool", bufs=1))
        ptp = top.enter_context(tc.tile_pool(name="ptp", bufs=48))
        spool = top.enter_context(tc.tile_pool(name="spool", bufs=1))
        osbp = top.enter_context(tc.tile_pool(name="osbp", bufs=8))
        rpool = top.enter_context(tc.tile_pool(name="rpool", bufs=4))
        outp = top.enter_context(tc.tile_pool(name="outp", bufs=16))
        ps_big = top.enter_context(tc.tile_pool(name="ps_big", bufs=4, space="PSUM"))
        ps_v = top.enter_context(tc.tile_pool(name="ps_v", bufs=2, space="PSUM"))
        ps_o = top.enter_context(tc.tile_pool(name="ps_o", bufs=1, space="PSUM"))
        ps_z = top.enter_context(tc.tile_pool(name="ps_z", bufs=1, space="PSUM"))

        # ---- consts: one packed DMA + bmask (Pool SWDGE queue: keeps the
        # HWDGE pipe clear for the x stream) ----
        cp = consts.tile([128, 44], f32r, tag="cp", name="cp")
        dmap(cp[:], cpack[:, :].bitcast(f32r))
        bm = consts.tile([8, 128], f32r, tag="bm", name="bm")
        dmap(bm[:], bmask.bitcast(f32r))
        gm = cp[:, 0:8]
        gm0 = cp[:, 28:36]
        gm75 = cp[:, 36:44]
        gam4 = cp[:, 8:12].bitcast(f32)
        bet4 = cp[:, 12:16].bitcast(f32)
        hqc = [cp[:, 16 + i:17 + i].bitcast(f32) for i in range(NCT)]
        bvc = [cp[:, 20 + i:21 + i].bitcast(f32) for i in range(NCT)]
        boc = [cp[:, 24 + i:25 + i].bitcast(f32) for i in range(NCT)]

        epst = consts.tile([128, 1], f32, tag="epst", name="epst")
        nc.vector.memset(epst[:], EPS)
        ebias = consts.tile([128, 1], f32, tag="ebias", name="ebias")
        nc.vector.memset(ebias[:], EBIAS)
        # z-ones are 16.0: they exactly cancel the x16 pre-scale on wv8
        # (kept out of e4m3 subnormal range), since o/z is scale-invariant
        ones8 = consts.tile([128, 2, 128], f8, tag="ones8", name="ones8")
        nc.vector.memset(ones8[:, :, :], 16.0)

        # prime the PE p-state clock: one tiny DR matmul right at t~0 so the
        # >3us ramp has elapsed by the time the real matmul stream starts
        prm = ps_z.tile([128, 2], f32, tag="zt", name="prm")
        nc.tensor.matmul(prm[:], ones8[:, :, 0:128], ones8[:, :, 0:2],
                         start=True, stop=True, perf_mode=DR)

        # ---- x resident first (h-major so fp8-pack column blocks complete
        # early). Stats chase the DMA: DVE runs bn_stats for ci 1-3 while
        # ACT covers ci0 with a fused fp8-cast+sum pass plus a Square+sum
        # pass (its xp tiles ride along for free); Pool casts the rest.
        # The group-sum matmul accumulates per ci as aggregates land. ----
        x_t = [[xpool.tile([128, 1024], bf16, tag=f"x{ci}_{h}",
                           name=f"x{ci}_{h}") for h in range(4)]
               for ci in range(NCT)]
        xp = [xq8p.tile([128, 2, N], f8, tag=f"xp{s}", name=f"xp{s}")
              for s in range(2)]
        statsAll = spool.tile([128, 8], f32r, tag="stA", name="statsAll")
        mvt = [spool.tile([128, 2], f32, tag=f"mv{i}", name=f"mv{i}")
               for i in range(NCT)]
        stats3 = [spool.tile([128, 8, 6], f32, tag=f"st3{i}", name=f"st3{i}")
                  for i in range(1, NCT)]
        aS = spool.tile([128, 4, 2], f32r, tag="aS", name="aS")
        aB = spool.tile([128, 2, 2], f32r, tag="aB", name="aB")
        sqscr = spool.tile([128, 1024], bf16, tag="sqscr", name="sqscr")
        # one PSUM accumulation region per ci: overlapping accumulation
        # groups in one bank are illegal, and the four group-sum streams
        # overlap in time. ps_o / ps_v are idle during the prologue.
        ssums = [ps_o.tile([8, 2], f32, tag="o", name="ss0"),
                 ps_v.tile([8, 2], f32, tag="v", name="ss1"),
                 ps_v.tile([8, 2], f32, tag="v", name="ss2"),
                 ps_z.tile([8, 2], f32, tag="zt", name="ss3")]
        sg = spool.tile([8, 8], f32r, tag="sg", name="sg")
        vneg = spool.tile([8, 4], f32, tag="vneg", name="vneg")
        for h in range(4):
            for ci in (0, 1, 2, 3):
                dma(x_t[ci][h][:],
                    x[ci * 128:(ci + 1) * 128, h * 1024:(h + 1) * 1024])
                dst = xp[ci // 2][:, ci % 2, h * 1024:(h + 1) * 1024]
                if ci == 0:
                    with nc.allow_low_precision(reason="f32r group sums"):
                        nc.scalar.activation(dst, x_t[ci][h][:], AF.Copy,
                                             accum_out=aS[:, h, 0:1])
                        nc.scalar.activation(sqscr[:], x_t[ci][h][:],
                                             AF.Square,
                                             accum_out=aS[:, h, 1:2])
                    # raw-sum group aggregation rides the PE with a mask
                    # pre-scaled by 1/(16*4096); no DVE combine ops at all
                    nc.tensor.matmul(ssums[0][:, :], gm0, aS[:, h, :],
                                     start=(h == 0), stop=(h == 3))
                    if h == 3:
                        nc.vector.tensor_copy(sg[:, 0:2], ssums[0][:, :])
                        nc.vector.scalar_tensor_tensor(
                            out=vneg[:, 0:1], in0=sg[:, 0:1],
                            scalar=sg[:, 0:1], in1=sg[:, 1:2],
                            op0=MUL, op1=mybir.AluOpType.subtract)
                    continue
                if ci in (1, 2) and h == 0:
                    # the h0 tiles of ci1/ci2 ride ACT too, easing the DVE
                    # bn_stats backlog; their raw sums fold into ssum via gm0
                    # while the bn path for h1-3 gets a 3/4-scaled mask
                    with nc.allow_low_precision(reason="f32r group sums"):
                        nc.scalar.activation(dst, x_t[ci][h][:], AF.Copy,
                                             accum_out=aB[:, ci - 1, 0:1])
                        nc.scalar.activation(sqscr[:], x_t[ci][h][:],
                                             AF.Square,
                                             accum_out=aB[:, ci - 1, 1:2])
                    nc.tensor.matmul(ssums[ci][:, :], gm0,
                                     aB[:, ci - 1, :],
                                     start=True, stop=False)
                    continue
                for k in range(2):
                    nc.vector.bn_stats(
                        out=stats3[ci - 1][:, h * 2 + k, :],
                        in_=x_t[ci][h][:, k * 512:(k + 1) * 512])
                nc.gpsimd.tensor_copy(dst, x_t[ci][h][:])
                if h == 3:
                    mv = mvt[ci]
                    in3 = (stats3[ci - 1][:, 2:8, :] if ci in (1, 2)
                           else stats3[ci - 1][:, :, :])
                    nc.vector.bn_aggr(out=mv[:], in_=in3)
                    nc.vector.tensor_copy(statsAll[:, 2 * ci:2 * ci + 1],
                                          mv[:, 0:1])
                    # E[x^2] = mean^2 + var in one fused op
                    nc.vector.scalar_tensor_tensor(
                        out=statsAll[:, 2 * ci + 1:2 * ci + 2], in0=mv[:, 0:1],
                        scalar=mv[:, 0:1], in1=mv[:, 1:2], op0=MUL, op1=ADD)
                    nc.tensor.matmul(ssums[ci][:, :],
                                     gm75 if ci in (1, 2) else gm,
                                     statsAll[:, 2 * ci:2 * ci + 2],
                                     start=(ci not in (1, 2)), stop=True)
                    nc.vector.tensor_copy(sg[:, 2 * ci:2 * ci + 2],
                                          ssums[ci][:, :])
                    # -var = mean^2 - E[x^2], fused per ci as its slice lands
                    nc.vector.scalar_tensor_tensor(
                        out=vneg[:, ci:ci + 1], in0=sg[:, 2 * ci:2 * ci + 1],
                        scalar=sg[:, 2 * ci:2 * ci + 1],
                        in1=sg[:, 2 * ci + 1:2 * ci + 2],
                        op0=MUL, op1=mybir.AluOpType.subtract)

        # ---- weights (land right after x on the HBM queue; single DMA
        # each via a strided dram view) ----
        wqt = wpool.tile([128, 4, C], bf16, tag="wqt", name="wqt")
        dma(wqt[:, :, :], wqk.rearrange('(c p) m -> p c m', p=128))
        wvt = wpool.tile([128, 4, C], bf16, tag="wvt", name="wvt")
        dma(wvt[:, :, :], wvT.rearrange('(c p) m -> p c m', p=128))
        wot = wpool.tile([128, 4, C], bf16, tag="wot", name="wot")
        dma(wot[:, :, :], woT.rearrange('(c p) m -> p c m', p=128))
        wqk_t = [wqt[:, i, :] for i in range(NCT)]
        wv_t = [wvt[:, i, :] for i in range(NCT)]
        wo_t = [wot[:, i, :] for i in range(NCT)]

        # ---- P1: group stats -> per-channel scale s_col / shift t4 ----
        # per-ci rstd chains: ci0-2's stats land ~2us before ci3's, so
        # their sqrt -> reciprocal -> broadcast legs complete early and only
        # ci3's short chain stays on the critical path to the qk projection
        # reciprocal first (stays on DVE, back-to-back with the fused
        # variance) then sqrt(-1/x) on ACT: one fewer cross-engine hop per ci
        rcv = spool.tile([8, 4], f32, tag="rcv", name="rcv")
        pc = ps_z.tile([128, 8], f32, tag="zt", name="pc")
        for ci in range(NCT):
            # eps (1e-6 vs var~1) is far below the fp8 pipeline noise floor;
            # folding it out keeps the chain one DVE op shorter
            with nc.allow_low_precision(reason="exact DVE divide"):
                nc.vector.reciprocal(rcv[:, ci:ci + 1], vneg[:, ci:ci + 1])
                nc.scalar.activation(sg[:, 2 * ci + 1:2 * ci + 2],
                                     rcv[:, ci:ci + 1], AF.Sqrt, scale=-1.0)
            nc.tensor.matmul(pc[:, 2 * ci:2 * ci + 2], bm[:],
                             sg[:, 2 * ci:2 * ci + 2], start=True, stop=True)
        # wqk scaled, bf16 (fp8 here would square the score noise); the
        # scale comes straight from the PSUM broadcast
        wqk_s = [wpool.tile([128, C], bf16, tag=f"wqs{i}", name=f"wqs{i}")
                 for i in range(NCT)]
        for ci in range(NCT):
            nc.vector.tensor_scalar_mul(wqk_s[ci][:], wqk_t[ci],
                                        pc[:, 2 * ci + 1:2 * ci + 2])
        rsb4 = consts.tile([128, 4], f32, tag="rsb4", name="rsb4")
        nc.vector.tensor_copy(rsb4[:], pc[:, 1:8:2])
        # t4 = -gmean*rstd (the y-form shift; beta terms are host-folded)
        t4 = consts.tile([128, 4], bf16, tag="t4", name="t4")
        nc.vector.scalar_tensor_tensor(out=t4[:], in0=pc[:, 0:8:2],
                                       scalar=-1.0,
                                       in1=rsb4[:], op0=MUL, op1=MUL)
        s64_4 = consts.tile([128, 4], f32, tag="s64_4", name="s64_4")
        nc.vector.scalar_tensor_tensor(out=s64_4[:], in0=rsb4[:], scalar=QKS,
                                       in1=gam4, op0=MUL, op1=MUL)
        s16_4 = consts.tile([128, 4], f32, tag="s16_4", name="s16_4")
        nc.vector.tensor_scalar_mul(s16_4[:], rsb4[:], 16.0)

        # ---- qk projection per i-block (bf16) + fp8 cast; the mt-sliced
        # PSUM lets casts chase the accumulation ----
        qk8 = [[qkp.tile([128, 2, 512], f8, tag=f"qk8_{s}_{ih}",
                         name=f"qk8_{s}_{ih}") for ih in range(2)]
               for s in range(2)]
        us4 = consts.tile([128, 4], f32, tag="us4", name="us4")
        su64_4 = consts.tile([128, 4], f32, tag="su64_4", name="su64_4")
        bvt4 = consts.tile([128, 4], bf16, tag="bvt4", name="bvt4")
        bos4 = consts.tile([128, 4], f32, tag="bos4", name="bos4")

        def gen_qk(ic, folds=False):
            qk_ps = [ps_big.tile([128, 2, 256], f32, tag="big",
                                 name=f"qkps{ic}_{half}") for half in range(2)]
            def cast_mt(mt):
                # ic0 casts on ACT (idle during the prologue; DVE still owns
                # the stats chain); later ics on DVE (truncating, but the
                # error headroom covers it) keeping ACT clear for exp
                dst = qk8[mt // 2][ic // 2][:, mt % 2,
                                            (ic % 2) * 256:(ic % 2) * 256 + 256]
                if ic == 0 and mt < 2:
                    nc.scalar.activation(
                        dst, qk_ps[mt // 2][:, mt % 2, :], AF.Identity,
                        scale=s64_4[:, mt:mt + 1], bias=su64_4[:, mt:mt + 1])
                else:
                    nc.vector.tensor_scalar(
                        dst, qk_ps[mt // 2][:, mt % 2, :],
                        s64_4[:, mt:mt + 1], su64_4[:, mt:mt + 1],
                        op0=MUL, op1=ADD)

            for mt in range(NCT):
                qkps = qk_ps[mt // 2][:, mt % 2, :]
                m_sl = slice(mt * 128, (mt + 1) * 128)
                for ct in range(NCT):
                    nc.tensor.matmul(qkps, wqk_s[ct][:, m_sl],
                                     x_t[ct][0][:, ic * 256:ic * 256 + 256],
                                     start=(ct == 0), stop=(ct == NCT - 1))
                if folds and mt == 0:
                    # P2 bias folds ride the PE stream right after the first
                    # qk tile: u = wqk^T t + hq (per out-channel), then
                    # bvt = wv^T t + bv. su64 = (u) * 64*scol gates the casts.
                    pq = ps_z.tile([128, 4], f32, tag="zt", name="pq")
                    for ot in range(NCT):
                        o_sl = slice(ot * 128, (ot + 1) * 128)
                        for ci in range(NCT):
                            nc.tensor.matmul(pq[:, ot:ot + 1],
                                             wqk_t[ci][:, o_sl],
                                             t4[:, ci:ci + 1],
                                             start=(ci == 0),
                                             stop=(ci == NCT - 1))
                    nc.vector.tensor_add(us4[:], pq[:, :],
                                         cp[:, 16:20].bitcast(f32))
                    nc.vector.tensor_mul(su64_4[:], us4[:], s64_4[:])
                    pv = ps_z.tile([128, 4], f32, tag="zt", name="pv")
                    for ot in range(NCT):
                        o_sl = slice(ot * 128, (ot + 1) * 128)
                        for ci in range(NCT):
                            nc.tensor.matmul(pv[:, ot:ot + 1],
                                             wv_t[ci][:, o_sl],
                                             t4[:, ci:ci + 1],
                                             start=(ci == 0),
                                             stop=(ci == NCT - 1))
                    bvf = spool.tile([128, 4], f32, tag="bvf", name="bvf")
                    nc.vector.tensor_add(bvf[:], pv[:, :],
                                         cp[:, 20:24].bitcast(f32))
                    nc.vector.tensor_copy(bvt4[:], bvf[:])
                yield
            for mt in range(NCT):
                cast_mt(mt)
                yield

        # wv -> fp8 packed with GN scale (x16 against subnormals) folded;
        # DVE so neither Pool (x-pack) nor ACT (casts+exp) stalls V
        wv8 = [wpool.tile([128, 2, C], f8, tag=f"wv8_{s}", name=f"wv8_{s}")
               for s in range(2)]

        def emit_wv8():
            for ci in range(NCT):
                if ci < 2:
                    nc.scalar.activation(wv8[ci // 2][:, ci % 2, :],
                                         wv_t[ci], AF.Identity,
                                         scale=s16_4[:, ci:ci + 1])
                else:
                    nc.vector.tensor_scalar_mul(wv8[ci // 2][:, ci % 2, :],
                                                wv_t[ci], s16_4[:, ci:ci + 1])

        # wo -> fp8 packed (raw weights; output projection runs DR)
        wo8 = [wpool.tile([128, 2, C], f8, tag=f"wo8_{s}", name=f"wo8_{s}")
               for s in range(2)]

        def emit_wo8():
            for ci in range(NCT):
                nc.gpsimd.tensor_copy(wo8[ci // 2][:, ci % 2, :], wo_t[ci])

        def emit_bo_fold():
            pb = ps_z.tile([128, 4], f32, tag="zt", name="pb")
            for ot in range(NCT):
                o_sl = slice(ot * 128, (ot + 1) * 128)
                for ci in range(NCT):
                    nc.tensor.matmul(pb[:, ot:ot + 1], wo_t[ci][:, o_sl],
                                     bvt4[:, ci:ci + 1],
                                     start=(ci == 0), stop=(ci == NCT - 1))
            nc.vector.tensor_add(bos4[:], pb[:, :], cp[:, 24:28].bitcast(f32))

        # ---- V projection (fp8 DR) on the 2-bank ps_v ring, one jt per
        # tile, interleaved into the scores streams; drains on DVE ----
        v8 = [vpool.tile([128, 2, C], f8, tag=f"v8_{p}", name=f"v8_{p}")
              for p in range(16)]

        def emit_vjt(jt):
            vt = ps_v.tile([128, 512], f32, tag="v", name=f"vt{jt}")
            for ch2 in range(2):
                for s in range(2):
                    nc.tensor.matmul(
                        vt[:, ch2 * 256:ch2 * 256 + 256],
                        xp[s][:, :, jt * 128:jt * 128 + 128],
                        wv8[s][:, :, ch2 * 256:ch2 * 256 + 256],
                        start=(s == 0), stop=(s == 1), perf_mode=DR)
            # drains on DVE: ACT is the exp pacer and must stay clear. DVE
            # truncates fp8 casts but V noise is fp8-quantization-dominated.
            nc.vector.tensor_copy(v8[jt // 2][:, jt % 2, :], vt[:, :])

        # ---- main attention loop over i-blocks of 256 ----
        pt8 = {}
        pending = []

        def drain(n):
            done = 0
            while pending and done < n:
                try:
                    next(pending[0])
                    done += 1
                except StopIteration:
                    pending.pop(0)

        def emit_scores(ic, with_v=(), per_quad=0):
            pt8[ic] = []
            vq = {q: [] for q in range(16)}
            for i, jt in enumerate(with_v):
                vq[min(15, 2 + i * 16 // len(with_v))].append(jt)
            for q in range(16):
                sq = ps_big.tile([128, 2, 256], f32, tag="big",
                                 name=f"sq{ic}_{q}")
                for jq in range(2):
                    jt = q * 2 + jq
                    for s in range(2):
                        nc.tensor.matmul(
                            sq[:, jq, :],
                            xp[s][:, :, jt * 128:jt * 128 + 128],
                            qk8[s][ic // 2][:, :, (ic % 2) * 256:
                                            (ic % 2) * 256 + 256],
                            start=(s == 0), stop=(s == 1), perf_mode=DR)
                pt = ptp.tile([128, 2, 256], f8, tag="pt", name=f"pt{ic}_{q}")
                nc.scalar.activation(pt[:, :, :], sq[:, :, :], AF.Exp,
                                     scale=1.0 / QKS, bias=ebias[:])
                pt8[ic].append(pt)
                for jt in vq[q]:
                    emit_vjt(jt)
                drain(per_quad)

        Rs = {}

        def gen_z(ic):
            """z chain -> R, yielded per quad for PE interleaving."""
            zt = ps_z.tile([128, 512], f32, tag="zt", name=f"zt{ic}")
            for q in range(16):
                while len(pt8.get(ic, ())) <= q:
                    yield   # scores for this quad not emitted yet
                nc.tensor.matmul(zt[:, 0:256], ones8[:, :, :],
                                 pt8[ic][q][:, :, :],
                                 start=(q == 0), stop=(q == 15), perf_mode=DR)
                if q % 2 == 1:
                    yield
            R = rpool.tile([128, 256], f32, tag="R", name=f"R{ic}")
            with nc.allow_low_precision(reason="exact DVE divide"):
                nc.vector.reciprocal(R[:], zt[:, 0:256])
            Rs[ic] = R

        def gen_ofin(ic):
            """O accumulation -> normalize -> output proj -> residual+store.
            Tiles rotate over ps_o / ps_v (free after V-projection) for ring
            depth; z keeps ps_z."""
            while ic not in Rs:
                yield   # z chain for this i-block not finished emitting
            R = Rs[ic]
            osb8 = [osbp.tile([128, 2, 256], f8, tag="osb",
                              name=f"osb{ic}_{s}") for s in range(2)]
            for mt in range(NCT):
                pool = ps_o if mt % 2 == 0 else ps_v
                ot_ps = pool.tile([128, 512], f32,
                                  tag="o" if mt % 2 == 0 else "v",
                                  name=f"o{ic}_{mt}")
                for p in range(16):
                    while len(pt8.get(ic, ())) <= p:
                        yield   # this quad's scores not emitted yet
                    nc.tensor.matmul(ot_ps[:, 0:256],
                                     v8[p][:, :, mt * 128:mt * 128 + 128],
                                     pt8[ic][p][:, :, :],
                                     start=(p == 0), stop=(p == 15),
                                     perf_mode=DR)
                    if p % 4 == 3:
                        yield
                nc.vector.tensor_mul(osb8[mt // 2][:, mt % 2, :],
                                     ot_ps[:, 0:256], R[:])
            for ot in range(NCT):
                o_sl = slice(ot * 128, (ot + 1) * 128)
                pool = ps_o if ot % 2 == 0 else ps_v
                ft = pool.tile([128, 512], f32,
                               tag="o" if ot % 2 == 0 else "v",
                               name=f"f{ic}_{ot}")
                for s in range(2):
                    nc.tensor.matmul(ft[:, 0:256], wo8[s][:, :, o_sl],
                                     osb8[s][:, :, :], start=(s == 0),
                                     stop=(s == 1), perf_mode=DR)
                ot_sb = outp.tile([128, 256], f32, tag="outsb",
                                  name=f"ot{ic}_{ot}")
                n0 = ic * 256
                h, off = n0 // 1024, n0 % 1024
                nc.vector.scalar_tensor_tensor(
                    out=ot_sb[:], in0=ft[:, 0:256], scalar=bos4[:, ot:ot + 1],
                    in1=x_t[ot][h][:, off:off + 256],
                    op0=ADD, op1=ADD)
                # spread final stores across queues: the tail DMAs otherwise
                # serialize behind one engine's issue overhead
                eng = [dma, dmaa, dma, dmaa][ot] if ic == 3 else dma
                eng(out[o_sl, ic * 256:ic * 256 + 256], ot_sb[:])
                yield

        # --- software-pipelined emission; the O/fin part of stream ic drains
        # one stream later than its z part so every v8 write is emitted
        # before any consumer ---
        for _ in gen_qk(0, folds=True):
            pass
        emit_wv8()
        emit_wo8()
        for _ in gen_qk(1):
            pass
        emit_scores(0, with_v=range(0, 10), per_quad=2)
        emit_bo_fold()
        pending.append(gen_z(0))
        pending.append(gen_qk(2))
        emit_scores(1, with_v=range(10, 32), per_quad=4)
        pending.append(gen_ofin(0))
        pending.append(gen_qk(3))
        pending.append(gen_z(1))
        pending.append(gen_ofin(1))
        emit_scores(2, per_quad=7)
        pending.append(gen_z(2))
        pending.append(gen_z(3))
        pending.append(gen_ofin(2))
        pending.append(gen_ofin(3))
        emit_scores(3, per_quad=8)
        drain(10 ** 9)

    _legalize_waits(nc, mybir)
    return nc


def kernel(**inputs):
    import ml_dtypes
    import concourse.bass  # noqa: F401
    from concourse.bass_utils import run_bass_kernel_spmd

    bft = ml_dtypes.bfloat16
    x = np.asarray(inputs["x"], dtype=np.float32)
    gamma = np.asarray(inputs["gamma"], np.float32)
    beta = np.asarray(inputs["beta"], np.float32)
    wq = np.asarray(inputs["wq"], np.float32)
    bq = np.asarray(inputs["bq"], np.float32)
    wk = np.asarray(inputs["wk"], np.float32)
    wv = np.asarray(inputs["wv"], np.float32)
    wo = np.asarray(inputs["wo"], np.float32)
    bv = np.asarray(inputs["bv"], np.float32)
    bo = np.asarray(inputs["bo"], np.float32)

    Bb, Cc, H, W = x.shape
    scale = Cc ** (-0.5)
    xf = x.reshape(Bb, Cc, H * W)

    wqk_raw = scale * (wq.T @ wk)
    hq = scale * (wk.T @ bq) + wqk_raw.T @ beta   # [C] (+ beta fold)
    wqk_h = np.ascontiguousarray(wqk_raw * gamma[:, None]).astype(bft)
    bvh = bv + wv @ beta                          # beta fold for V
    wvT = np.ascontiguousarray((wv * gamma[None, :]).T).astype(bft)
    woT = np.ascontiguousarray(wo.T).astype(bft)

    cpack = np.zeros((128, 44), np.float32)
    for p in range(128):
        cpack[p, p // 16] = 1.0 / 16.0            # gmask (x 1/16)
        cpack[p, 28 + p // 16] = 1.0 / (16.0 * 4096.0)   # raw-sum gmask
        cpack[p, 36 + p // 16] = 0.75 / 16.0      # 3/4-weighted gmask (ci1)
    cpack[:, 8:12] = gamma.reshape(NCT, 128).T
    cpack[:, 12:16] = beta.reshape(NCT, 128).T
    cpack[:, 16:20] = hq.reshape(NCT, 128).T
    cpack[:, 20:24] = bvh.reshape(NCT, 128).T
    cpack[:, 24:28] = bo.reshape(NCT, 128).T
    bmask = np.zeros((8, 128), np.float32)
    for p in range(128):
        bmask[p // 16, p] = 1.0

    if "nc" not in _cache:
        _cache["nc"] = _build()
    nc = _cache["nc"]

    in_maps = []
    for core in range(8):
        b, qi = core // 4, core % 4
        xb = xf[b]
        # rotate columns so this core's query quarter sits at columns [0, NQ)
        xrot = np.ascontiguousarray(
            np.concatenate([xb[:, qi * NQ:], xb[:, :qi * NQ]],
                           axis=1)).astype(bft)
        in_maps.append({
            "x": xrot, "wqk": wqk_h, "wvT": wvT, "woT": woT,
            "cpack": cpack, "bmask": bmask,
        })

    res = run_bass_kernel_spmd(nc, in_maps, core_ids=list(range(8)))
    outf = np.empty((Bb, Cc, H * W), np.float32)
    for core in range(8):
        b, qi = core // 4, core % 4
        outf[b][:, qi * NQ:(qi + 1) * NQ] = res.results[core]["out"]
    return outf.reshape(Bb, Cc, H, W)
